# revision 35
# baseline (speedup 1.0000x reference)
"""Distributed Trainium2 kernel for nn_Attention_68719477187.

RoPE + causal GQA attention (B=2, S=2048, DIM=2048, 32 q heads / 8 kv heads,
head_dim 64) on 8 NeuronCores: DP=2 over batch x TP=4 over head groups.

Per core (b = core//4, G = core%4): 8 q heads / 2 kv heads of batch b.
  1. qkv.T = w{q,k,v}T.T @ x_b.T (contraction over model dim on partitions)
  2. RoPE applied in transposed layout; head_dim pre-permuted (evens, odds)
     on the host so rotation pairs become contiguous 32-partition blocks.
  3. scores.T tiles (k on partitions, q on free) -> exp (no max subtraction;
     scores are O(5) so fp32 exp is safe) -> causal mask by 0/1 multiply on
     the 128-wide diagonal block only -> AV matmul with a ones-column
     appended to V so the softmax denominator falls out of the same matmul.
  4. wo partial sums: each core contracts its OWN 512 attention dims
     against wo rows for ALL 2048 output cols (no AllGather needed), then a
     per-seq-chunk ReduceScatter(add) over each batch group of 4 cores
     hands rank G the final 512-dim output block G.  The RS result lands
     directly in the bf16 output parameter (host transposes/casts).

Scheduling: the attention inner loop is software-pipelined (scores run two
key-tiles ahead of the AV matmuls) and a filler queue interleaves wo blocks
of the previous chunk and qkv rows of the next chunk between attention
tiles, keeping the PE tensor engine continuously busy (full p-state clock)
while the ACT engine works through the softmax exps.

Compute in bf16 (fp32 PSUM accumulation), output bf16 (cast on host).
"""

import sys

if "/opt/trn_rl_repo" not in sys.path:
    sys.path.insert(0, "/opt/trn_rl_repo")

from collections import deque

import numpy as np
import ml_dtypes

from concourse import bacc, tile, mybir
from concourse.bass_utils import run_bass_kernel_spmd
from concourse.tile_rust import add_dep_helper

BF16 = ml_dtypes.bfloat16

S = 2048          # sequence length
D = 2048          # model dim
HD = 64           # head dim
NQL = 8           # local q heads
NKVL = 2          # local kv heads
QC = 512          # q chunk (matmul free dim)
NSC = S // QC     # 4 seq chunks
NKD = D // 128    # 16 contraction tiles
NKT = S // 128    # 16 key tiles
SCALE = HD ** -0.5

# attention sub-chunks: (qT chunk, col offset, width).  Chunk 3 is split
# 256+256 so the 3a ReduceScatter (21.5us) hides under sub-3b's ~26us of
# attention PE work, leaving only the small 3b RS after PE finishes.
SUBS = [(0, 0, QC), (1, 0, QC), (2, 0, QC), (3, 0, 256), (3, 256, 256)]

_NC = None


def _build(_no_cc=False):
    import os
    # filler-drain tuning knobs (ns of filler credit per attention tile,
    # filler unit cost, queue pressure threshold); fall back to defaults.
    try:
        cred_n, unit_ns, press = [int(v) for v in
                                  os.environ.get("KTUNE", "").split(",")]
    except ValueError:
        cred_n, unit_ns, press = 650, 850, 26
    nc = bacc.Bacc("TRN2", target_bir_lowering=False, debug=False, num_devices=8)
    BF = mybir.dt.bfloat16
    F32 = mybir.dt.float32
    EXP = mybir.ActivationFunctionType.Exp

    # all inputs host-staged to per-partition-contiguous SBUF layouts so DMA
    # descriptor counts stay low (SEQ dispatch cost ~ descriptors)
    xS = nc.declare_dram_parameter("xS", [NSC, 128, NKD, QC], BF, isOutput=False)
    wqS = nc.declare_dram_parameter("wqS", [128, NKD, 512], BF, isOutput=False)
    wkS = nc.declare_dram_parameter("wkS", [128, NKD, 128], BF, isOutput=False)
    wvS = nc.declare_dram_parameter("wvS", [128, NKD, 128], BF, isOutput=False)
    woS = nc.declare_dram_parameter("woS", [128, 4, D], BF, isOutput=False)
    cosS = nc.declare_dram_parameter("cosS", [128, S], BF, isOutput=False)
    sinS = nc.declare_dram_parameter("sinS", [128, S], BF, isOutput=False)
    mask = nc.declare_dram_parameter("mask", [128, 2, 128], BF, isOutput=False)
    # flat output: 4 chunks of (512 out-dims, 512 seq) bf16, host reassembles
    out = nc.declare_dram_parameter("out", [NSC * 512 * QC], BF, isOutput=True)

    with tile.TileContext(nc) as tc:
        with (
            nc.allow_low_precision(reason="bf16 rope/softmax-normalize chain"),
            tc.tile_pool(name="wpool", bufs=1) as wpool,
            tc.tile_pool(name="pers", bufs=1) as pers,
            tc.tile_pool(name="dram", bufs=1, space="DRAM") as dram,
            tc.tile_pool(name="xpool", bufs=12) as xpool,
            tc.tile_pool(name="rtmp", bufs=2) as rtmp,
            tc.tile_pool(name="ppool", bufs=4) as ppool,
            tc.tile_pool(name="npool", bufs=2) as npool,
            tc.tile_pool(name="guard", bufs=1) as guard,
            tc.tile_pool(name="apool", bufs=2) as apool,
            tc.tile_pool(name="opool", bufs=4) as opool,
            tc.tile_pool(name="gps", bufs=2, space="PSUM") as gps,
            tc.tile_pool(name="stps", bufs=2, space="PSUM") as stps,
            tc.tile_pool(name="avps", bufs=2, space="PSUM") as avps,
        ):
            # ---- persistent weights / constants (one 3D DMA each) ----
            wq_sb = [wpool.tile([128, NKD // 4, 512], BF, name=f"wq_sb{h}",
                                tag=f"wq_sb{h}") for h in range(4)]
            wk_sb = wpool.tile([128, NKD, 128], BF, name="wk_sb", tag="wk_sb")
            wv_sb = wpool.tile([128, NKD, 128], BF, name="wv_sb", tag="wv_sb")
            wo_sb = wpool.tile([128, 4, D], BF, name="wo_sb", tag="wo_sb")
            cos_sb = wpool.tile([128, S], BF, name="cos_sb", tag="cos_sb")
            sin_sb = wpool.tile([128, S], BF, name="sin_sb", tag="sin_sb")
            mask_sb = wpool.tile([128, 2, 128], BF, name="mask_sb", tag="mask_sb")

            # ---- persistent activations ----
            qT = [[pers.tile([128, QC], BF, name=f"qT_{rt}_{sc}", tag=f"qT_{rt}_{sc}")
                   for sc in range(NSC)] for rt in range(4)]
            kdup = [[pers.tile([128, QC], BF, name=f"kd_{j}_{sc}", tag=f"kd_{j}_{sc}")
                     for sc in range(NSC)] for j in range(NKVL)]
            vaug = [pers.tile([128, 2, 65], BF, name=f"va_{kt}", tag=f"va_{kt}")
                    for kt in range(NKT)]
            # per-sub ReduceScatter staging: (2048 out-dims, qn seq); the
            # collective may not write IO tensors, so it lands in rs_out and
            # later DMAs move it to the out param.  Chunks 0-2 are whole;
            # chunk 3 is two 256-wide halves (subs 3 and 4).
            rs_w = [QC, QC, QC, 256, 256]
            rs_in = [dram.tile([D, rs_w[i]], BF, name=f"rs_in_{i}")
                     for i in range(5)]
            rs_out = [dram.tile([512, rs_w[i]], BF, name=f"rs_out_{i}")
                      for i in range(5)]
            # flat out offsets per rs block
            rs_off = [0, 512 * QC, 2 * 512 * QC, 3 * 512 * QC,
                      3 * 512 * QC + 512 * 256]

            # hoist x loads so the (in-order) SP DMA queue never stalls them;
            # wq / x chunk 0 are loaded in halves so the first matmuls start
            # after ~1MB of DMA instead of 4MB.
            xts = {}

            # the scheduler reorders same-queue DMAs by its own heuristics
            # (observed: chunk-1 x pushed behind wo/cos/sin, starving the
            # chunk-1 qkv fillers until ~40us); chain the startup preloads
            # with nosync deps so they issue in emission order
            prev_dma = [None]

            def sdma(dst, src):
                d = nc.sync.dma_start(dst, src)
                if prev_dma[0] is not None:
                    add_dep_helper(d.ins, prev_dma[0], False,
                                   reason="preload DMA order")
                prev_dma[0] = d.ins
                return d

            def load_x(sc, chain=False):
                parts = []
                for h in range(4):
                    xt = xpool.tile([128, NKD // 4, QC], BF, name="xt", tag="xt")
                    if chain:
                        sdma(xt[:], xS[sc, :, h * 4:(h + 1) * 4, :])
                    else:
                        nc.sync.dma_start(xt[:], xS[sc, :, h * 4:(h + 1) * 4, :])
                    parts.append(xt)
                xts[sc] = parts

            # k/v-first startup: wk+wv are small (0.5MB each) and the k row
            # and v tiles contract over x as its quarters arrive, so PE
            # starts ~4us in; wq quarters follow and the q rows run last
            sdma(wk_sb[:], wkS[:])
            xts[0] = []
            for h in range(4):
                xt = xpool.tile([128, NKD // 4, QC], BF, name="xt", tag="xt")
                sdma(xt[:], xS[0, :, h * 4:(h + 1) * 4, :])
                xts[0].append(xt)
                if h == 0:
                    sdma(wv_sb[:], wvS[:])
            sdma(cos_sb[:, 0:QC], cosS[:, 0:QC])
            sdma(sin_sb[:, 0:QC], sinS[:, 0:QC])
            sdma(mask_sb[:], mask[:])
            for h in range(4):
                sdma(wq_sb[h][:], wqS[:, h * 4:(h + 1) * 4, :])
            for sc in range(1, NSC - 1):
                load_x(sc, chain=True)
                sdma(cos_sb[:, sc * QC:(sc + 1) * QC],
                     cosS[:, sc * QC:(sc + 1) * QC])
                sdma(sin_sb[:, sc * QC:(sc + 1) * QC],
                     sinS[:, sc * QC:(sc + 1) * QC])
            sdma(cos_sb[:, 3 * QC:], cosS[:, 3 * QC:])
            sdma(sin_sb[:, 3 * QC:], sinS[:, 3 * QC:])
            sdma(wo_sb[:], woS[:])
            # scratch for the trailing dummy collective (teardown guard)
            cg_in = dram.tile([128, 2], BF, name="cg_in")
            cg_out = dram.tile([512, 2], BF, name="cg_out")
            nc.sync.dma_start(cg_in[:], mask[:, 0, 0:2])

            # ---- qkv projection + rope ----
            # each row is one atomic unit: its PSUM accumulation (tag "gp")
            # must not interleave with other "gp" allocations (buffer reuse
            # would clobber the accumulation in flight)
            def qkv_row(sc, rt):
                xt = xts[sc]
                ps = gps.tile([128, QC], F32, name="gp", tag="gp")
                for kd in range(NKD):
                    lhsT = (wq_sb[kd // 4][:, kd % 4, rt * 128:(rt + 1) * 128]
                            if rt < 4 else wk_sb[:, kd, :])
                    nc.tensor.matmul(ps[:], lhsT, xt[kd // 4][:, kd % 4, :],
                                     start=(kd == 0), stop=(kd == NKD - 1))
                cslice = cos_sb[:, sc * QC:(sc + 1) * QC]
                sslice = sin_sb[:, sc * QC:(sc + 1) * QC]
                # rope in fp32 (bf16 only at the final q/k write):
                # out = raw*cos + swap32(raw)*sin_signed
                raw = rtmp.tile([128, QC], BF, name="raw", tag="raw")
                nc.vector.tensor_copy(raw[:], ps[:])
                rot = rtmp.tile([128, QC], BF, name="rot", tag="rot")
                for b32 in range(4):
                    src = (b32 ^ 1) * 32
                    nc.gpsimd.tensor_copy(rot[b32 * 32:(b32 + 1) * 32, :],
                                          raw[src:src + 32, :])
                t1 = rtmp.tile([128, QC], BF, name="t1", tag="t1")
                nc.vector.tensor_mul(t1[:], raw[:], cslice)
                nc.vector.tensor_mul(rot[:], rot[:], sslice)
                if rt < 4:
                    nc.vector.tensor_add(qT[rt][sc][:], t1[:], rot[:])
                else:
                    kr = rtmp.tile([128, QC], BF, name="kr", tag="kr")
                    nc.vector.tensor_add(kr[:], t1[:], rot[:])
                    for j in range(NKVL):
                        src = kr[j * 64:(j + 1) * 64, :]
                        nc.gpsimd.tensor_copy(kdup[j][sc][0:64, :], src)
                        nc.gpsimd.tensor_copy(kdup[j][sc][64:128, :], src)

            def v_tt(sc, tt):
                # V computed directly in natural (seq, dim) orientation:
                # lhsT = x.T seq-slice, rhs = wv.T -> out (seq, 2*64) + ones
                xt = xts[sc]
                kt = sc * 4 + tt
                vp = gps.tile([128, QC], F32, name="gp", tag="gp")
                for kd in range(NKD):
                    nc.tensor.matmul(vp[:, 0:128],
                                     xt[kd // 4][:, kd % 4, tt * 128:(tt + 1) * 128],
                                     wv_sb[:, kd, :],
                                     start=(kd == 0), stop=(kd == NKD - 1))
                for j in range(NKVL):
                    nc.vector.tensor_copy(vaug[kt][:, j, 0:64],
                                          vp[:, j * 64:(j + 1) * 64])
                    nc.gpsimd.memset(vaug[kt][:, j, 64:65], 1.0)

            def qkv_units(sc):
                if sc == 0:
                    # chunk 0 runs k and v first: their weights load first
                    # and attention phase 0 needs kdup/vaug anyway
                    units = [lambda: qkv_row(0, 4)]
                    units += [lambda tt=tt: v_tt(0, tt) for tt in range(4)]
                    units += [lambda rt=rt: qkv_row(0, rt) for rt in range(4)]
                    return units
                units = [lambda rt=rt: qkv_row(sc, rt) for rt in range(5)]
                units += [lambda tt=tt: v_tt(sc, tt) for tt in range(4)]
                return units

            # ---- wo partial + per-chunk ReduceScatter ----
            atiles = {}

            def wo_oc(i, oc):
                qc, q0, qn = SUBS[i]
                ps = gps.tile([128, QC], F32, name="gp", tag="gp")
                at = atiles[i]
                for j in range(4):
                    nc.tensor.matmul(ps[:, 0:qn],
                                     wo_sb[:, j, oc * 128:(oc + 1) * 128],
                                     at[:, j, 0:qn],
                                     start=(j == 0), stop=(j == 3))
                ot = opool.tile([128, QC], BF, name="ot", tag="ot")
                # PSUM is only readable by DVE/ACT; split the copies between
                # them (DVE-heavy: ACT is the softmax bottleneck)
                if oc % 3 == 2:
                    nc.scalar.copy(ot[:, 0:qn], ps[:, 0:qn])
                else:
                    nc.vector.tensor_copy(ot[:, 0:qn], ps[:, 0:qn])
                nc.sync.dma_start(rs_in[i][oc * 128:(oc + 1) * 128, 0:qn],
                                  ot[:, 0:qn])

            def rs_chunk(i):
                if _no_cc:
                    # sim-only: local copy instead of the collective, to
                    # measure compute-schedule quality without the cost
                    # model's collective pricing.
                    nc.gpsimd.dma_start(rs_out[i][:], rs_in[i][0:512, :])
                else:
                    nc.gpsimd.collective_compute(
                        "ReduceScatter", mybir.AluOpType.add,
                        replica_groups=[[0, 1, 2, 3], [4, 5, 6, 7]],
                        ins=[rs_in[i].opt()],
                        outs=[rs_out[i].opt()])
                if i != 4:
                    # the out-DMA is NOT issued here: on the in-order ACT
                    # queue it would wait for the RS to complete and
                    # head-of-line block the softmax exps for ~15-30us,
                    # starving PE.  out_dma(i) is instead queued as filler
                    # one chunk later, when the RS has long completed.
                    return
                if not _no_cc:
                    # teardown guard for the final block, whose reader was
                    # the only one ever observed stale: (1) ~22us of chained
                    # ACT busywork ahead of the out-DMA on the in-order ACT
                    # queue delays the rs_out[4] read well past the RS
                    # (delayed readers -- blocks 0-3 -- always see correct
                    # data); (2) a trailing dummy collective after the DMA
                    # ensures the real RS is never the last collective at
                    # program teardown.
                    d1 = guard.tile([128, 2048], F32, name="d1", tag="d1")
                    d2 = guard.tile([128, 2048], F32, name="d2", tag="d2")
                    g = nc.scalar.copy(d1[:], cos_sb[:])
                    if last_exp[0] is not None:
                        # without this gate the scheduler hoists the guard
                        # busywork into the attention region, starving the
                        # ACT engine exactly where exps are latency-critical
                        add_dep_helper(g.ins, last_exp[0], False,
                                       reason="guard chain after final exp")
                    for _ in range(6):
                        nc.scalar.copy(d2[:], d1[:])
                        nc.scalar.copy(d1[:], d2[:])
                out_dma(3)
                out_dma(4)
                if not _no_cc:
                    nc.gpsimd.collective_compute(
                        "AllGather", mybir.AluOpType.bypass,
                        replica_groups=[[0, 1, 2, 3], [4, 5, 6, 7]],
                        ins=[cg_in.opt()], outs=[cg_out.opt()])

            # most recently emitted exp activation; out_dma gates on it so
            # the tile scheduler cannot hoist the DMA (whose embedded
            # collective-semaphore wait would head-of-line block the ACT
            # queue) ahead of the attention exps emitted before it
            last_exp = [None]

            def out_dma(i):
                d = nc.scalar.dma_start(
                    out[rs_off[i]:rs_off[i] + 512 * rs_w[i]], rs_out[i][:])
                if last_exp[0] is not None:
                    add_dep_helper(d.ins, last_exp[0], False,
                                   reason="delay out-dma past attention exps")

            def wo_units(i):
                units = [lambda oc=oc: wo_oc(i, oc) for oc in range(16)]
                units.append(lambda i=i: rs_chunk(i))
                return units

            # ---- filler queue ----
            fq = deque()

            def drain(k):
                for _ in range(k):
                    if not fq:
                        return
                    fq.popleft()()

            def drain_all():
                while fq:
                    fq.popleft()()

            # ---- attention, software-pipelined over key tiles ----
            def attn_phase(i):
                qc, q0, qn = SUBS[i]
                gqs = qc * QC + q0          # 128-aligned global q start
                t0 = gqs // 128             # first diagonal key tile
                nkt = (gqs + qn) // 128     # causal: key tiles up to sub end
                # attention outputs staged in one tile: (128, rt, seq-chunk)
                atile = apool.tile([128, 4, QC], BF, name="atile", tag="atile")
                atiles[i] = atile
                for rt in range(4):  # head pair (2rt, 2rt+1); shared kv head
                    j = rt // 2
                    avs = [avps.tile([65, QC], F32, name="av", tag="av")
                           for _ in range(2)]
                    ptiles = {}

                    def emit_score(kt, rt=rt, j=j, ptiles=ptiles):
                        kb = (kt % 4) * 128
                        # diagonal k-tiles only need q columns >= 128*m
                        m = kt - t0
                        qo = 128 * m if m > 0 else 0
                        n = qn - qo
                        st = stps.tile([128, 2, QC], F32, name="st", tag="st")
                        for half in range(2):
                            # operands at partition base 64*half -> the two
                            # K=64 matmuls run in different PE row groups
                            lo, hi = half * 64, half * 64 + 64
                            nc.tensor.matmul(st[:, half, 0:n],
                                             kdup[j][kt // 4][lo:hi, kb:kb + 128],
                                             qT[rt][qc][lo:hi, q0 + qo:q0 + qn],
                                             start=True, stop=True)
                        p = ppool.tile([128, 2, QC], BF, name="p", tag="p")
                        e = nc.scalar.activation(p[:, :, 0:n], st[:, :, 0:n],
                                                 EXP, scale=SCALE)
                        last_exp[0] = e.ins if hasattr(e, "ins") else e
                        if m >= 0:
                            # only the first 128 q-columns of a diagonal tile
                            # intersect the triangle; the rest is unmasked
                            nc.vector.tensor_mul(p[:, :, 0:128], p[:, :, 0:128],
                                                 mask_sb[:])
                        ptiles[kt] = (p, qo, n)

                    def emit_av(kt, rt=rt, j=j, ptiles=ptiles, nkt=nkt):
                        p, qo, n = ptiles.pop(kt)
                        for half in range(2):
                            nc.tensor.matmul(avs[half][:, qo:qn],
                                             vaug[kt][:, j, :],
                                             p[:, half, 0:n],
                                             start=(kt == 0), stop=(kt == nkt - 1))

                    emit_score(0)
                    if nkt > 1:
                        emit_score(1)
                    credit = 0.0
                    for kt in range(nkt):
                        credit += cred_n * (qn - (128 * (kt - t0) if kt > t0 else 0)) / QC
                        while fq and (credit >= unit_ns or len(fq) > press):
                            fq.popleft()()
                            credit -= unit_ns
                        if kt + 2 < nkt:
                            emit_score(kt + 2)
                        emit_av(kt)
                    for half in range(2):
                        av = avs[half]
                        # drain PSUM->SBUF first so the av PSUM bank frees
                        # after one copy instead of after the whole
                        # recip->broadcast->mul chain (next rt's first AV
                        # matmul reuses the bank)
                        avsb = npool.tile([65, QC], BF, name="avsb",
                                          tag="avsb")
                        nc.vector.tensor_copy(avsb[:, 0:qn], av[:, 0:qn])
                        recip = npool.tile([1, QC], BF, name="recip", tag="recip")
                        nc.vector.reciprocal(recip[:, 0:qn], avsb[64:65, 0:qn])
                        rb = npool.tile([64, QC], BF, name="rb", tag="rb")
                        nc.gpsimd.partition_broadcast(rb[:, 0:qn], recip[:, 0:qn])
                        nc.vector.tensor_mul(
                            atile[half * 64:(half + 1) * 64, rt, 0:qn],
                            avsb[0:64, 0:qn], rb[:, 0:qn])
                    drain(1)

            # ---- schedule ----
            for u in qkv_units(0):
                u()
            fq.extend(qkv_units(1))
            attn_phase(0)
            fq.append(lambda: load_x(3))
            fq.extend(wo_units(0))
            fq.extend(qkv_units(2))
            attn_phase(1)
            fq.extend(wo_units(1))
            fq.extend(qkv_units(3))
            attn_phase(2)
            fq.extend(wo_units(2))
            fq.append(lambda: out_dma(0))
            attn_phase(3)
            # sub-3a wo runs immediately (not as paced filler) so its RS
            # dispatches before sub-3b attention and hides under it
            drain_all()
            for u in wo_units(3):
                u()
            fq.append(lambda: out_dma(1))
            attn_phase(4)
            drain_all()
            out_dma(2)
            for u in wo_units(4):
                u()

    nc.compile()
    return nc


def _get_nc():
    global _NC
    if _NC is None:
        _NC = _build()
    return _NC


def _prepare_in_maps(x, freqs_cis, wqkv, wo):
    x = np.asarray(x)
    freqs_cis = np.asarray(freqs_cis)
    wqkv = np.asarray(wqkv)
    wo = np.asarray(wo)

    perm = np.concatenate([np.arange(0, HD, 2), np.arange(1, HD, 2)])
    cos = np.ascontiguousarray(freqs_cis[:, :, 0].T)  # (32, S)
    sin = np.ascontiguousarray(freqs_cis[:, :, 1].T)
    cosS = np.ascontiguousarray(np.concatenate([cos, cos, cos, cos], axis=0),
                                dtype=np.float32).astype(BF16)
    sinS = np.ascontiguousarray(np.concatenate([-sin, sin, -sin, sin], axis=0),
                                dtype=np.float32).astype(BF16)
    p_i = np.arange(128)[:, None]
    f_i = np.arange(128)[None, :]
    tri = (f_i >= p_i)
    mask = np.stack([tri, tri], axis=1).astype(BF16)

    def stage(wt, nkd=NKD):
        # (Dc, C) with Dc = nkd*128 -> (128, nkd, C), per-partition contiguous
        return np.ascontiguousarray(
            wt.reshape(nkd, 128, wt.shape[1]).transpose(1, 0, 2)).astype(BF16)

    xSs = []
    for b in range(2):
        xt = x[b].T  # (D, S)
        xs = xt.reshape(NKD, 128, NSC, QC).transpose(2, 1, 0, 3)
        xSs.append(np.ascontiguousarray(xs).astype(BF16))

    in_maps = []
    for c in range(8):
        b, G = c // 4, c % 4
        qrows = np.concatenate([(8 * G + h) * HD + perm for h in range(NQL)])
        krows = np.concatenate([D + (2 * G + j) * HD + perm for j in range(NKVL)])
        vrows = np.concatenate([D + 512 + (2 * G + j) * HD + np.arange(HD)
                                for j in range(NKVL)])
        in_maps.append({
            "xS": xSs[b],
            "wqS": stage(wqkv[qrows, :].T),
            "wkS": stage(wqkv[krows, :].T),
            "wvS": stage(wqkv[vrows, :].T),
            "woS": stage(np.ascontiguousarray(wo[:, 512 * G:512 * (G + 1)].T),
                         nkd=4),
            "cosS": cosS,
            "sinS": sinS,
            "mask": mask,
        })
    return in_maps


def kernel(x, freqs_cis, wqkv, wo, _trace=False):
    in_maps = _prepare_in_maps(x, freqs_cis, wqkv, wo)
    res = run_bass_kernel_spmd(_get_nc(), in_maps, core_ids=list(range(8)),
                               trace=_trace)

    outf = np.empty((2, S, D), np.float32)
    # 5 rs blocks of (512 dims, w seq): seq chunks 0-2 full, chunk 3 halved
    blocks = [(0, 512), (512, 512), (1024, 512), (1536, 256), (1792, 256)]
    for c in range(8):
        b, G = c // 4, c % 4
        flat = np.asarray(res.results[c]["out"])
        off = 0
        for s0, w in blocks:
            blk = flat[off:off + 512 * w].reshape(512, w)
            outf[b, s0:s0 + w, 512 * G:512 * (G + 1)] = blk.T.astype(np.float32)
            off += 512 * w
    if _trace:
        kernel.last_exec_time_ns = res.exec_time_ns
        kernel.last_results = res
    return outf



# revision 36
# speedup vs baseline: 1.0106x; 1.0106x over previous
"""Distributed Trainium2 kernel for nn_Attention_68719477187.

RoPE + causal GQA attention (B=2, S=2048, DIM=2048, 32 q heads / 8 kv heads,
head_dim 64) on 8 NeuronCores: DP=2 over batch x TP=4 over head groups.

Per core (b = core//4, G = core%4): 8 q heads / 2 kv heads of batch b.
  1. qkv.T = w{q,k,v}T.T @ x_b.T (contraction over model dim on partitions)
  2. RoPE applied in transposed layout; head_dim pre-permuted (evens, odds)
     on the host so rotation pairs become contiguous 32-partition blocks.
  3. scores.T tiles (k on partitions, q on free) -> exp (no max subtraction;
     scores are O(5) so fp32 exp is safe) -> causal mask by 0/1 multiply on
     the 128-wide diagonal block only -> AV matmul with a ones-column
     appended to V so the softmax denominator falls out of the same matmul.
  4. wo partial sums: each core contracts its OWN 512 attention dims
     against wo rows for ALL 2048 output cols (no AllGather needed), then a
     per-seq-chunk ReduceScatter(add) over each batch group of 4 cores
     hands rank G the final 512-dim output block G.  The RS result lands
     directly in the bf16 output parameter (host transposes/casts).

Scheduling: the attention inner loop is software-pipelined (scores run two
key-tiles ahead of the AV matmuls) and a filler queue interleaves wo blocks
of the previous chunk and qkv rows of the next chunk between attention
tiles, keeping the PE tensor engine continuously busy (full p-state clock)
while the ACT engine works through the softmax exps.

Compute in bf16 (fp32 PSUM accumulation), output bf16 (cast on host).
"""

import sys

if "/opt/trn_rl_repo" not in sys.path:
    sys.path.insert(0, "/opt/trn_rl_repo")

from collections import deque

import numpy as np
import ml_dtypes

from concourse import bacc, tile, mybir
from concourse.bass_utils import run_bass_kernel_spmd
from concourse.tile_rust import add_dep_helper

BF16 = ml_dtypes.bfloat16

S = 2048          # sequence length
D = 2048          # model dim
HD = 64           # head dim
NQL = 8           # local q heads
NKVL = 2          # local kv heads
QC = 512          # q chunk (matmul free dim)
NSC = S // QC     # 4 seq chunks
NKD = D // 128    # 16 contraction tiles
NKT = S // 128    # 16 key tiles
SCALE = HD ** -0.5

# attention sub-chunks: (qT chunk, col offset, width).  Chunk 3 is split
# 256+256 so the 3a ReduceScatter (21.5us) hides under sub-3b's ~26us of
# attention PE work, leaving only the small 3b RS after PE finishes.
SUBS = [(0, 0, QC), (1, 0, QC), (2, 0, QC), (3, 0, 256), (3, 256, 256)]

_NC = None


def _build(_no_cc=False):
    import os
    # filler-drain tuning knobs (ns of filler credit per attention tile,
    # filler unit cost, queue pressure threshold); fall back to defaults.
    try:
        cred_n, unit_ns, press = [int(v) for v in
                                  os.environ.get("KTUNE", "").split(",")]
    except ValueError:
        cred_n, unit_ns, press = 650, 850, 26
    nc = bacc.Bacc("TRN2", target_bir_lowering=False, debug=False, num_devices=8)
    BF = mybir.dt.bfloat16
    F32 = mybir.dt.float32
    EXP = mybir.ActivationFunctionType.Exp

    # all inputs host-staged to per-partition-contiguous SBUF layouts so DMA
    # descriptor counts stay low (SEQ dispatch cost ~ descriptors)
    xS = nc.declare_dram_parameter("xS", [NSC, 128, NKD, QC], BF, isOutput=False)
    wqS = nc.declare_dram_parameter("wqS", [128, NKD, 512], BF, isOutput=False)
    wkS = nc.declare_dram_parameter("wkS", [128, NKD, 128], BF, isOutput=False)
    wvS = nc.declare_dram_parameter("wvS", [128, NKD, 128], BF, isOutput=False)
    woS = nc.declare_dram_parameter("woS", [128, 4, D], BF, isOutput=False)
    cosS = nc.declare_dram_parameter("cosS", [128, S], BF, isOutput=False)
    sinS = nc.declare_dram_parameter("sinS", [128, S], BF, isOutput=False)
    mask = nc.declare_dram_parameter("mask", [128, 2, 128], BF, isOutput=False)
    # flat output: 4 chunks of (512 out-dims, 512 seq) bf16, host reassembles
    out = nc.declare_dram_parameter("out", [NSC * 512 * QC], BF, isOutput=True)

    with tile.TileContext(nc) as tc:
        with (
            nc.allow_low_precision(reason="bf16 rope/softmax-normalize chain"),
            tc.tile_pool(name="wpool", bufs=1) as wpool,
            tc.tile_pool(name="pers", bufs=1) as pers,
            tc.tile_pool(name="dram", bufs=1, space="DRAM") as dram,
            tc.tile_pool(name="xpool", bufs=12) as xpool,
            tc.tile_pool(name="rtmp", bufs=4) as rtmp,
            tc.tile_pool(name="ppool", bufs=4) as ppool,
            tc.tile_pool(name="npool", bufs=2) as npool,
            tc.tile_pool(name="guard", bufs=1) as guard,
            tc.tile_pool(name="apool", bufs=2) as apool,
            tc.tile_pool(name="opool", bufs=4) as opool,
            tc.tile_pool(name="gps", bufs=2, space="PSUM") as gps,
            tc.tile_pool(name="stps", bufs=2, space="PSUM") as stps,
            tc.tile_pool(name="avps", bufs=2, space="PSUM") as avps,
        ):
            # ---- persistent weights / constants (one 3D DMA each) ----
            wq_sb = [wpool.tile([128, NKD // 4, 512], BF, name=f"wq_sb{h}",
                                tag=f"wq_sb{h}") for h in range(4)]
            wk_sb = wpool.tile([128, NKD, 128], BF, name="wk_sb", tag="wk_sb")
            wv_sb = wpool.tile([128, NKD, 128], BF, name="wv_sb", tag="wv_sb")
            wo_sb = wpool.tile([128, 4, D], BF, name="wo_sb", tag="wo_sb")
            cos_sb = wpool.tile([128, S], BF, name="cos_sb", tag="cos_sb")
            sin_sb = wpool.tile([128, S], BF, name="sin_sb", tag="sin_sb")
            mask_sb = wpool.tile([128, 2, 128], BF, name="mask_sb", tag="mask_sb")

            # ---- persistent activations ----
            qT = [[pers.tile([128, QC], BF, name=f"qT_{rt}_{sc}", tag=f"qT_{rt}_{sc}")
                   for sc in range(NSC)] for rt in range(4)]
            kdup = [[pers.tile([128, QC], BF, name=f"kd_{j}_{sc}", tag=f"kd_{j}_{sc}")
                     for sc in range(NSC)] for j in range(NKVL)]
            vaug = [pers.tile([128, 2, 65], BF, name=f"va_{kt}", tag=f"va_{kt}")
                    for kt in range(NKT)]
            # per-sub ReduceScatter staging: (2048 out-dims, qn seq); the
            # collective may not write IO tensors, so it lands in rs_out and
            # later DMAs move it to the out param.  Chunks 0-2 are whole;
            # chunk 3 is two 256-wide halves (subs 3 and 4).
            rs_w = [QC, QC, QC, 256, 256]
            rs_in = [dram.tile([D, rs_w[i]], BF, name=f"rs_in_{i}")
                     for i in range(5)]
            rs_out = [dram.tile([512, rs_w[i]], BF, name=f"rs_out_{i}")
                      for i in range(5)]
            # flat out offsets per rs block
            rs_off = [0, 512 * QC, 2 * 512 * QC, 3 * 512 * QC,
                      3 * 512 * QC + 512 * 256]

            # hoist x loads so the (in-order) SP DMA queue never stalls them;
            # wq / x chunk 0 are loaded in halves so the first matmuls start
            # after ~1MB of DMA instead of 4MB.
            xts = {}

            # the scheduler reorders same-queue DMAs by its own heuristics
            # (observed: chunk-1 x pushed behind wo/cos/sin, starving the
            # chunk-1 qkv fillers until ~40us); chain the startup preloads
            # with nosync deps so they issue in emission order
            prev_dma = [None]

            def sdma(dst, src):
                d = nc.sync.dma_start(dst, src)
                if prev_dma[0] is not None:
                    add_dep_helper(d.ins, prev_dma[0], False,
                                   reason="preload DMA order")
                prev_dma[0] = d.ins
                return d

            def load_x(sc, chain=False):
                parts = []
                for h in range(4):
                    xt = xpool.tile([128, NKD // 4, QC], BF, name="xt", tag="xt")
                    if chain:
                        sdma(xt[:], xS[sc, :, h * 4:(h + 1) * 4, :])
                    else:
                        nc.sync.dma_start(xt[:], xS[sc, :, h * 4:(h + 1) * 4, :])
                    parts.append(xt)
                xts[sc] = parts

            # k/v-first startup: wk+wv are small (0.5MB each) and the k row
            # and v tiles contract over x as its quarters arrive, so PE
            # starts ~4us in; wq quarters follow and the q rows run last
            sdma(wk_sb[:], wkS[:])
            xts[0] = []
            for h in range(4):
                xt = xpool.tile([128, NKD // 4, QC], BF, name="xt", tag="xt")
                sdma(xt[:], xS[0, :, h * 4:(h + 1) * 4, :])
                xts[0].append(xt)
                if h == 0:
                    sdma(wv_sb[:], wvS[:])
            sdma(cos_sb[:, 0:QC], cosS[:, 0:QC])
            sdma(sin_sb[:, 0:QC], sinS[:, 0:QC])
            sdma(mask_sb[:], mask[:])
            for h in range(4):
                sdma(wq_sb[h][:], wqS[:, h * 4:(h + 1) * 4, :])
            for sc in range(1, NSC - 1):
                load_x(sc, chain=True)
                sdma(cos_sb[:, sc * QC:(sc + 1) * QC],
                     cosS[:, sc * QC:(sc + 1) * QC])
                sdma(sin_sb[:, sc * QC:(sc + 1) * QC],
                     sinS[:, sc * QC:(sc + 1) * QC])
            sdma(cos_sb[:, 3 * QC:], cosS[:, 3 * QC:])
            sdma(sin_sb[:, 3 * QC:], sinS[:, 3 * QC:])
            sdma(wo_sb[:], woS[:])
            # scratch for the trailing dummy collective (teardown guard)
            cg_in = dram.tile([128, 2], BF, name="cg_in")
            cg_out = dram.tile([512, 2], BF, name="cg_out")
            nc.sync.dma_start(cg_in[:], mask[:, 0, 0:2])

            # ---- qkv projection + rope ----
            # each row is one atomic unit: its PSUM accumulation (tag "gp")
            # must not interleave with other "gp" allocations (buffer reuse
            # would clobber the accumulation in flight)
            def qkv_row(sc, rt):
                xt = xts[sc]
                ps = gps.tile([128, QC], F32, name="gp", tag="gp")
                for kd in range(NKD):
                    lhsT = (wq_sb[kd // 4][:, kd % 4, rt * 128:(rt + 1) * 128]
                            if rt < 4 else wk_sb[:, kd, :])
                    nc.tensor.matmul(ps[:], lhsT, xt[kd // 4][:, kd % 4, :],
                                     start=(kd == 0), stop=(kd == NKD - 1))
                cslice = cos_sb[:, sc * QC:(sc + 1) * QC]
                sslice = sin_sb[:, sc * QC:(sc + 1) * QC]
                # rope in fp32 (bf16 only at the final q/k write):
                # out = raw*cos + swap32(raw)*sin_signed
                raw = rtmp.tile([128, QC], BF, name="raw", tag="raw")
                nc.vector.tensor_copy(raw[:], ps[:])
                rot = rtmp.tile([128, QC], BF, name="rot", tag="rot")
                for b32 in range(4):
                    src = (b32 ^ 1) * 32
                    nc.gpsimd.tensor_copy(rot[b32 * 32:(b32 + 1) * 32, :],
                                          raw[src:src + 32, :])
                t1 = rtmp.tile([128, QC], BF, name="t1", tag="t1")
                nc.vector.tensor_mul(t1[:], raw[:], cslice)
                nc.vector.tensor_mul(rot[:], rot[:], sslice)
                if rt < 4:
                    nc.vector.tensor_add(qT[rt][sc][:], t1[:], rot[:])
                else:
                    kr = rtmp.tile([128, QC], BF, name="kr", tag="kr")
                    nc.vector.tensor_add(kr[:], t1[:], rot[:])
                    for j in range(NKVL):
                        src = kr[j * 64:(j + 1) * 64, :]
                        nc.gpsimd.tensor_copy(kdup[j][sc][0:64, :], src)
                        nc.gpsimd.tensor_copy(kdup[j][sc][64:128, :], src)

            def v_tt(sc, tt):
                # V computed directly in natural (seq, dim) orientation:
                # lhsT = x.T seq-slice, rhs = wv.T -> out (seq, 2*64) + ones
                xt = xts[sc]
                kt = sc * 4 + tt
                vp = gps.tile([128, QC], F32, name="gp", tag="gp")
                for kd in range(NKD):
                    nc.tensor.matmul(vp[:, 0:128],
                                     xt[kd // 4][:, kd % 4, tt * 128:(tt + 1) * 128],
                                     wv_sb[:, kd, :],
                                     start=(kd == 0), stop=(kd == NKD - 1))
                for j in range(NKVL):
                    nc.vector.tensor_copy(vaug[kt][:, j, 0:64],
                                          vp[:, j * 64:(j + 1) * 64])
                    nc.gpsimd.memset(vaug[kt][:, j, 64:65], 1.0)

            def qkv_units(sc):
                if sc == 0:
                    # chunk 0 runs k and v first: their weights load first
                    # and attention phase 0 needs kdup/vaug anyway
                    units = [lambda: qkv_row(0, 4)]
                    units += [lambda tt=tt: v_tt(0, tt) for tt in range(4)]
                    units += [lambda rt=rt: qkv_row(0, rt) for rt in range(4)]
                    return units
                units = [lambda rt=rt: qkv_row(sc, rt) for rt in range(5)]
                units += [lambda tt=tt: v_tt(sc, tt) for tt in range(4)]
                return units

            # ---- wo partial + per-chunk ReduceScatter ----
            atiles = {}

            def wo_oc(i, oc):
                qc, q0, qn = SUBS[i]
                ps = gps.tile([128, QC], F32, name="gp", tag="gp")
                at = atiles[i]
                for j in range(4):
                    nc.tensor.matmul(ps[:, 0:qn],
                                     wo_sb[:, j, oc * 128:(oc + 1) * 128],
                                     at[:, j, 0:qn],
                                     start=(j == 0), stop=(j == 3))
                ot = opool.tile([128, QC], BF, name="ot", tag="ot")
                # PSUM is only readable by DVE/ACT; split the copies between
                # them (DVE-heavy: ACT is the softmax bottleneck)
                if oc % 3 == 2:
                    nc.scalar.copy(ot[:, 0:qn], ps[:, 0:qn])
                else:
                    nc.vector.tensor_copy(ot[:, 0:qn], ps[:, 0:qn])
                nc.sync.dma_start(rs_in[i][oc * 128:(oc + 1) * 128, 0:qn],
                                  ot[:, 0:qn])

            def rs_chunk(i):
                if _no_cc:
                    # sim-only: local copy instead of the collective, to
                    # measure compute-schedule quality without the cost
                    # model's collective pricing.
                    nc.gpsimd.dma_start(rs_out[i][:], rs_in[i][0:512, :])
                else:
                    nc.gpsimd.collective_compute(
                        "ReduceScatter", mybir.AluOpType.add,
                        replica_groups=[[0, 1, 2, 3], [4, 5, 6, 7]],
                        ins=[rs_in[i].opt()],
                        outs=[rs_out[i].opt()])
                if i != 4:
                    # the out-DMA is NOT issued here: on the in-order ACT
                    # queue it would wait for the RS to complete and
                    # head-of-line block the softmax exps for ~15-30us,
                    # starving PE.  out_dma(i) is instead queued as filler
                    # one chunk later, when the RS has long completed.
                    return
                if not _no_cc:
                    # teardown guard for the final block, whose reader was
                    # the only one ever observed stale: (1) ~22us of chained
                    # ACT busywork ahead of the out-DMA on the in-order ACT
                    # queue delays the rs_out[4] read well past the RS
                    # (delayed readers -- blocks 0-3 -- always see correct
                    # data); (2) a trailing dummy collective after the DMA
                    # ensures the real RS is never the last collective at
                    # program teardown.
                    d1 = guard.tile([128, 2048], F32, name="d1", tag="d1")
                    d2 = guard.tile([128, 2048], F32, name="d2", tag="d2")
                    g = nc.scalar.copy(d1[:], cos_sb[:])
                    if last_exp[0] is not None:
                        # without this gate the scheduler hoists the guard
                        # busywork into the attention region, starving the
                        # ACT engine exactly where exps are latency-critical
                        add_dep_helper(g.ins, last_exp[0], False,
                                       reason="guard chain after final exp")
                    for _ in range(6):
                        nc.scalar.copy(d2[:], d1[:])
                        nc.scalar.copy(d1[:], d2[:])
                out_dma(3)
                out_dma(4)
                if not _no_cc:
                    nc.gpsimd.collective_compute(
                        "AllGather", mybir.AluOpType.bypass,
                        replica_groups=[[0, 1, 2, 3], [4, 5, 6, 7]],
                        ins=[cg_in.opt()], outs=[cg_out.opt()])

            # most recently emitted exp activation; out_dma gates on it so
            # the tile scheduler cannot hoist the DMA (whose embedded
            # collective-semaphore wait would head-of-line block the ACT
            # queue) ahead of the attention exps emitted before it
            last_exp = [None]

            def out_dma(i):
                d = nc.scalar.dma_start(
                    out[rs_off[i]:rs_off[i] + 512 * rs_w[i]], rs_out[i][:])
                if last_exp[0] is not None:
                    add_dep_helper(d.ins, last_exp[0], False,
                                   reason="delay out-dma past attention exps")

            def wo_units(i):
                units = [lambda oc=oc: wo_oc(i, oc) for oc in range(16)]
                units.append(lambda i=i: rs_chunk(i))
                return units

            # ---- filler queue ----
            fq = deque()

            def drain(k):
                for _ in range(k):
                    if not fq:
                        return
                    fq.popleft()()

            def drain_all():
                while fq:
                    fq.popleft()()

            # ---- attention, software-pipelined over key tiles ----
            def attn_phase(i):
                qc, q0, qn = SUBS[i]
                gqs = qc * QC + q0          # 128-aligned global q start
                t0 = gqs // 128             # first diagonal key tile
                nkt = (gqs + qn) // 128     # causal: key tiles up to sub end
                # attention outputs staged in one tile: (128, rt, seq-chunk)
                atile = apool.tile([128, 4, QC], BF, name="atile", tag="atile")
                atiles[i] = atile
                for rt in range(4):  # head pair (2rt, 2rt+1); shared kv head
                    j = rt // 2
                    avs = [avps.tile([65, QC], F32, name="av", tag="av")
                           for _ in range(2)]
                    ptiles = {}

                    def emit_score(kt, rt=rt, j=j, ptiles=ptiles):
                        kb = (kt % 4) * 128
                        # diagonal k-tiles only need q columns >= 128*m
                        m = kt - t0
                        qo = 128 * m if m > 0 else 0
                        n = qn - qo
                        st = stps.tile([128, 2, QC], F32, name="st", tag="st")
                        for half in range(2):
                            # operands at partition base 64*half -> the two
                            # K=64 matmuls run in different PE row groups
                            lo, hi = half * 64, half * 64 + 64
                            nc.tensor.matmul(st[:, half, 0:n],
                                             kdup[j][kt // 4][lo:hi, kb:kb + 128],
                                             qT[rt][qc][lo:hi, q0 + qo:q0 + qn],
                                             start=True, stop=True)
                        p = ppool.tile([128, 2, QC], BF, name="p", tag="p")
                        e = nc.scalar.activation(p[:, :, 0:n], st[:, :, 0:n],
                                                 EXP, scale=SCALE)
                        last_exp[0] = e.ins if hasattr(e, "ins") else e
                        if m >= 0:
                            # only the first 128 q-columns of a diagonal tile
                            # intersect the triangle; the rest is unmasked
                            nc.vector.tensor_mul(p[:, :, 0:128], p[:, :, 0:128],
                                                 mask_sb[:])
                        ptiles[kt] = (p, qo, n)

                    def emit_av(kt, rt=rt, j=j, ptiles=ptiles, nkt=nkt):
                        p, qo, n = ptiles.pop(kt)
                        for half in range(2):
                            nc.tensor.matmul(avs[half][:, qo:qn],
                                             vaug[kt][:, j, :],
                                             p[:, half, 0:n],
                                             start=(kt == 0), stop=(kt == nkt - 1))

                    emit_score(0)
                    if nkt > 1:
                        emit_score(1)
                    credit = 0.0
                    for kt in range(nkt):
                        credit += cred_n * (qn - (128 * (kt - t0) if kt > t0 else 0)) / QC
                        while fq and (credit >= unit_ns or len(fq) > press):
                            fq.popleft()()
                            credit -= unit_ns
                        if kt + 2 < nkt:
                            emit_score(kt + 2)
                        emit_av(kt)
                    for half in range(2):
                        av = avs[half]
                        # drain PSUM->SBUF first so the av PSUM bank frees
                        # after one copy instead of after the whole
                        # recip->broadcast->mul chain (next rt's first AV
                        # matmul reuses the bank)
                        avsb = npool.tile([65, QC], BF, name="avsb",
                                          tag="avsb")
                        nc.vector.tensor_copy(avsb[:, 0:qn], av[:, 0:qn])
                        recip = npool.tile([1, QC], BF, name="recip", tag="recip")
                        nc.vector.reciprocal(recip[:, 0:qn], avsb[64:65, 0:qn])
                        rb = npool.tile([64, QC], BF, name="rb", tag="rb")
                        nc.gpsimd.partition_broadcast(rb[:, 0:qn], recip[:, 0:qn])
                        nc.vector.tensor_mul(
                            atile[half * 64:(half + 1) * 64, rt, 0:qn],
                            avsb[0:64, 0:qn], rb[:, 0:qn])
                    drain(1)

            # ---- schedule ----
            for u in qkv_units(0):
                u()
            fq.extend(qkv_units(1))
            attn_phase(0)
            fq.append(lambda: load_x(3))
            fq.extend(wo_units(0))
            fq.extend(qkv_units(2))
            attn_phase(1)
            fq.extend(wo_units(1))
            fq.extend(qkv_units(3))
            attn_phase(2)
            fq.extend(wo_units(2))
            fq.append(lambda: out_dma(0))
            attn_phase(3)
            # sub-3a wo runs immediately (not as paced filler) so its RS
            # dispatches before sub-3b attention and hides under it
            drain_all()
            for u in wo_units(3):
                u()
            fq.append(lambda: out_dma(1))
            attn_phase(4)
            drain_all()
            out_dma(2)
            for u in wo_units(4):
                u()

    nc.compile()
    return nc


def _get_nc():
    global _NC
    if _NC is None:
        _NC = _build()
    return _NC


def _prepare_in_maps(x, freqs_cis, wqkv, wo):
    x = np.asarray(x)
    freqs_cis = np.asarray(freqs_cis)
    wqkv = np.asarray(wqkv)
    wo = np.asarray(wo)

    perm = np.concatenate([np.arange(0, HD, 2), np.arange(1, HD, 2)])
    cos = np.ascontiguousarray(freqs_cis[:, :, 0].T)  # (32, S)
    sin = np.ascontiguousarray(freqs_cis[:, :, 1].T)
    cosS = np.ascontiguousarray(np.concatenate([cos, cos, cos, cos], axis=0),
                                dtype=np.float32).astype(BF16)
    sinS = np.ascontiguousarray(np.concatenate([-sin, sin, -sin, sin], axis=0),
                                dtype=np.float32).astype(BF16)
    p_i = np.arange(128)[:, None]
    f_i = np.arange(128)[None, :]
    tri = (f_i >= p_i)
    mask = np.stack([tri, tri], axis=1).astype(BF16)

    def stage(wt, nkd=NKD):
        # (Dc, C) with Dc = nkd*128 -> (128, nkd, C), per-partition contiguous
        return np.ascontiguousarray(
            wt.reshape(nkd, 128, wt.shape[1]).transpose(1, 0, 2)).astype(BF16)

    xSs = []
    for b in range(2):
        xt = x[b].T  # (D, S)
        xs = xt.reshape(NKD, 128, NSC, QC).transpose(2, 1, 0, 3)
        xSs.append(np.ascontiguousarray(xs).astype(BF16))

    in_maps = []
    for c in range(8):
        b, G = c // 4, c % 4
        qrows = np.concatenate([(8 * G + h) * HD + perm for h in range(NQL)])
        krows = np.concatenate([D + (2 * G + j) * HD + perm for j in range(NKVL)])
        vrows = np.concatenate([D + 512 + (2 * G + j) * HD + np.arange(HD)
                                for j in range(NKVL)])
        in_maps.append({
            "xS": xSs[b],
            "wqS": stage(wqkv[qrows, :].T),
            "wkS": stage(wqkv[krows, :].T),
            "wvS": stage(wqkv[vrows, :].T),
            "woS": stage(np.ascontiguousarray(wo[:, 512 * G:512 * (G + 1)].T),
                         nkd=4),
            "cosS": cosS,
            "sinS": sinS,
            "mask": mask,
        })
    return in_maps


def kernel(x, freqs_cis, wqkv, wo, _trace=False):
    in_maps = _prepare_in_maps(x, freqs_cis, wqkv, wo)
    res = run_bass_kernel_spmd(_get_nc(), in_maps, core_ids=list(range(8)),
                               trace=_trace)

    outf = np.empty((2, S, D), np.float32)
    # 5 rs blocks of (512 dims, w seq): seq chunks 0-2 full, chunk 3 halved
    blocks = [(0, 512), (512, 512), (1024, 512), (1536, 256), (1792, 256)]
    for c in range(8):
        b, G = c // 4, c % 4
        flat = np.asarray(res.results[c]["out"])
        off = 0
        for s0, w in blocks:
            blk = flat[off:off + 512 * w].reshape(512, w)
            outf[b, s0:s0 + w, 512 * G:512 * (G + 1)] = blk.T.astype(np.float32)
            off += 512 * w
    if _trace:
        kernel.last_exec_time_ns = res.exec_time_ns
        kernel.last_results = res
    return outf



# revision 37
# speedup vs baseline: 1.0112x; 1.0007x over previous
"""Distributed Trainium2 kernel for nn_Attention_68719477187.

RoPE + causal GQA attention (B=2, S=2048, DIM=2048, 32 q heads / 8 kv heads,
head_dim 64) on 8 NeuronCores: DP=2 over batch x TP=4 over head groups.

Per core (b = core//4, G = core%4): 8 q heads / 2 kv heads of batch b.
  1. qkv.T = w{q,k,v}T.T @ x_b.T (contraction over model dim on partitions)
  2. RoPE applied in transposed layout; head_dim pre-permuted (evens, odds)
     on the host so rotation pairs become contiguous 32-partition blocks.
  3. scores.T tiles (k on partitions, q on free) -> exp (no max subtraction;
     scores are O(5) so fp32 exp is safe) -> causal mask by 0/1 multiply on
     the 128-wide diagonal block only -> AV matmul with a ones-column
     appended to V so the softmax denominator falls out of the same matmul.
  4. wo partial sums: each core contracts its OWN 512 attention dims
     against wo rows for ALL 2048 output cols (no AllGather needed), then a
     per-seq-chunk ReduceScatter(add) over each batch group of 4 cores
     hands rank G the final 512-dim output block G.  The RS result lands
     directly in the bf16 output parameter (host transposes/casts).

Scheduling: the attention inner loop is software-pipelined (scores run two
key-tiles ahead of the AV matmuls) and a filler queue interleaves wo blocks
of the previous chunk and qkv rows of the next chunk between attention
tiles, keeping the PE tensor engine continuously busy (full p-state clock)
while the ACT engine works through the softmax exps.

Compute in bf16 (fp32 PSUM accumulation), output bf16 (cast on host).
"""

import sys

if "/opt/trn_rl_repo" not in sys.path:
    sys.path.insert(0, "/opt/trn_rl_repo")

from collections import deque

import numpy as np
import ml_dtypes

from concourse import bacc, tile, mybir
from concourse.bass_utils import run_bass_kernel_spmd
from concourse.tile_rust import add_dep_helper

BF16 = ml_dtypes.bfloat16

S = 2048          # sequence length
D = 2048          # model dim
HD = 64           # head dim
NQL = 8           # local q heads
NKVL = 2          # local kv heads
QC = 512          # q chunk (matmul free dim)
NSC = S // QC     # 4 seq chunks
NKD = D // 128    # 16 contraction tiles
NKT = S // 128    # 16 key tiles
SCALE = HD ** -0.5

# attention sub-chunks: (qT chunk, col offset, width).  Chunk 3 is split
# 256+256 so the 3a ReduceScatter (21.5us) hides under sub-3b's ~26us of
# attention PE work, leaving only the small 3b RS after PE finishes.
SUBS = [(0, 0, QC), (1, 0, QC), (2, 0, QC), (3, 0, 256), (3, 256, 256)]

_NC = None


def _build(_no_cc=False):
    import os
    # filler-drain tuning knobs (ns of filler credit per attention tile,
    # filler unit cost, queue pressure threshold); fall back to defaults.
    try:
        cred_n, unit_ns, press = [int(v) for v in
                                  os.environ.get("KTUNE", "").split(",")]
    except ValueError:
        cred_n, unit_ns, press = 650, 850, 26
    nc = bacc.Bacc("TRN2", target_bir_lowering=False, debug=False, num_devices=8)
    BF = mybir.dt.bfloat16
    F32 = mybir.dt.float32
    EXP = mybir.ActivationFunctionType.Exp

    # all inputs host-staged to per-partition-contiguous SBUF layouts so DMA
    # descriptor counts stay low (SEQ dispatch cost ~ descriptors)
    xS = nc.declare_dram_parameter("xS", [NSC, 128, NKD, QC], BF, isOutput=False)
    wqS = nc.declare_dram_parameter("wqS", [128, NKD, 512], BF, isOutput=False)
    wkS = nc.declare_dram_parameter("wkS", [128, NKD, 128], BF, isOutput=False)
    wvS = nc.declare_dram_parameter("wvS", [128, NKD, 128], BF, isOutput=False)
    woS = nc.declare_dram_parameter("woS", [128, 4, D], BF, isOutput=False)
    cosS = nc.declare_dram_parameter("cosS", [128, S], BF, isOutput=False)
    sinS = nc.declare_dram_parameter("sinS", [128, S], BF, isOutput=False)
    mask = nc.declare_dram_parameter("mask", [128, 2, 128], BF, isOutput=False)
    # flat output: 4 chunks of (512 out-dims, 512 seq) bf16, host reassembles
    out = nc.declare_dram_parameter("out", [NSC * 512 * QC], BF, isOutput=True)

    with tile.TileContext(nc) as tc:
        with (
            nc.allow_low_precision(reason="bf16 rope/softmax-normalize chain"),
            tc.tile_pool(name="wpool", bufs=1) as wpool,
            tc.tile_pool(name="pers", bufs=1) as pers,
            tc.tile_pool(name="dram", bufs=1, space="DRAM") as dram,
            tc.tile_pool(name="xpool", bufs=12) as xpool,
            tc.tile_pool(name="rtmp", bufs=4) as rtmp,
            tc.tile_pool(name="ppool", bufs=6) as ppool,
            tc.tile_pool(name="npool", bufs=2) as npool,
            tc.tile_pool(name="guard", bufs=1) as guard,
            tc.tile_pool(name="apool", bufs=2) as apool,
            tc.tile_pool(name="opool", bufs=4) as opool,
            tc.tile_pool(name="gps", bufs=2, space="PSUM") as gps,
            tc.tile_pool(name="stps", bufs=2, space="PSUM") as stps,
            tc.tile_pool(name="avps", bufs=2, space="PSUM") as avps,
        ):
            # ---- persistent weights / constants (one 3D DMA each) ----
            wq_sb = [wpool.tile([128, NKD // 4, 512], BF, name=f"wq_sb{h}",
                                tag=f"wq_sb{h}") for h in range(4)]
            wk_sb = wpool.tile([128, NKD, 128], BF, name="wk_sb", tag="wk_sb")
            wv_sb = wpool.tile([128, NKD, 128], BF, name="wv_sb", tag="wv_sb")
            wo_sb = wpool.tile([128, 4, D], BF, name="wo_sb", tag="wo_sb")
            cos_sb = wpool.tile([128, S], BF, name="cos_sb", tag="cos_sb")
            sin_sb = wpool.tile([128, S], BF, name="sin_sb", tag="sin_sb")
            mask_sb = wpool.tile([128, 2, 128], BF, name="mask_sb", tag="mask_sb")

            # ---- persistent activations ----
            qT = [[pers.tile([128, QC], BF, name=f"qT_{rt}_{sc}", tag=f"qT_{rt}_{sc}")
                   for sc in range(NSC)] for rt in range(4)]
            kdup = [[pers.tile([128, QC], BF, name=f"kd_{j}_{sc}", tag=f"kd_{j}_{sc}")
                     for sc in range(NSC)] for j in range(NKVL)]
            vaug = [pers.tile([128, 2, 65], BF, name=f"va_{kt}", tag=f"va_{kt}")
                    for kt in range(NKT)]
            # per-sub ReduceScatter staging: (2048 out-dims, qn seq); the
            # collective may not write IO tensors, so it lands in rs_out and
            # later DMAs move it to the out param.  Chunks 0-2 are whole;
            # chunk 3 is two 256-wide halves (subs 3 and 4).
            rs_w = [QC, QC, QC, 256, 256]
            rs_in = [dram.tile([D, rs_w[i]], BF, name=f"rs_in_{i}")
                     for i in range(5)]
            rs_out = [dram.tile([512, rs_w[i]], BF, name=f"rs_out_{i}")
                      for i in range(5)]
            # flat out offsets per rs block
            rs_off = [0, 512 * QC, 2 * 512 * QC, 3 * 512 * QC,
                      3 * 512 * QC + 512 * 256]

            # hoist x loads so the (in-order) SP DMA queue never stalls them;
            # wq / x chunk 0 are loaded in halves so the first matmuls start
            # after ~1MB of DMA instead of 4MB.
            xts = {}

            # the scheduler reorders same-queue DMAs by its own heuristics
            # (observed: chunk-1 x pushed behind wo/cos/sin, starving the
            # chunk-1 qkv fillers until ~40us); chain the startup preloads
            # with nosync deps so they issue in emission order
            prev_dma = [None]

            def sdma(dst, src):
                d = nc.sync.dma_start(dst, src)
                if prev_dma[0] is not None:
                    add_dep_helper(d.ins, prev_dma[0], False,
                                   reason="preload DMA order")
                prev_dma[0] = d.ins
                return d

            def load_x(sc, chain=False):
                parts = []
                for h in range(4):
                    xt = xpool.tile([128, NKD // 4, QC], BF, name="xt", tag="xt")
                    if chain:
                        sdma(xt[:], xS[sc, :, h * 4:(h + 1) * 4, :])
                    else:
                        nc.sync.dma_start(xt[:], xS[sc, :, h * 4:(h + 1) * 4, :])
                    parts.append(xt)
                xts[sc] = parts

            # k/v-first startup: wk+wv are small (0.5MB each) and the k row
            # and v tiles contract over x as its quarters arrive, so PE
            # starts ~4us in; wq quarters follow and the q rows run last
            sdma(wk_sb[:], wkS[:])
            xts[0] = []
            for h in range(4):
                xt = xpool.tile([128, NKD // 4, QC], BF, name="xt", tag="xt")
                sdma(xt[:], xS[0, :, h * 4:(h + 1) * 4, :])
                xts[0].append(xt)
                if h == 0:
                    sdma(wv_sb[:], wvS[:])
            sdma(cos_sb[:, 0:QC], cosS[:, 0:QC])
            sdma(sin_sb[:, 0:QC], sinS[:, 0:QC])
            sdma(mask_sb[:], mask[:])
            for h in range(4):
                sdma(wq_sb[h][:], wqS[:, h * 4:(h + 1) * 4, :])
            for sc in range(1, NSC - 1):
                load_x(sc, chain=True)
                sdma(cos_sb[:, sc * QC:(sc + 1) * QC],
                     cosS[:, sc * QC:(sc + 1) * QC])
                sdma(sin_sb[:, sc * QC:(sc + 1) * QC],
                     sinS[:, sc * QC:(sc + 1) * QC])
            sdma(cos_sb[:, 3 * QC:], cosS[:, 3 * QC:])
            sdma(sin_sb[:, 3 * QC:], sinS[:, 3 * QC:])
            sdma(wo_sb[:], woS[:])
            # scratch for the trailing dummy collective (teardown guard)
            cg_in = dram.tile([128, 2], BF, name="cg_in")
            cg_out = dram.tile([512, 2], BF, name="cg_out")
            nc.sync.dma_start(cg_in[:], mask[:, 0, 0:2])

            # ---- qkv projection + rope ----
            # each row is one atomic unit: its PSUM accumulation (tag "gp")
            # must not interleave with other "gp" allocations (buffer reuse
            # would clobber the accumulation in flight)
            def qkv_row(sc, rt):
                xt = xts[sc]
                ps = gps.tile([128, QC], F32, name="gp", tag="gp")
                for kd in range(NKD):
                    lhsT = (wq_sb[kd // 4][:, kd % 4, rt * 128:(rt + 1) * 128]
                            if rt < 4 else wk_sb[:, kd, :])
                    nc.tensor.matmul(ps[:], lhsT, xt[kd // 4][:, kd % 4, :],
                                     start=(kd == 0), stop=(kd == NKD - 1))
                cslice = cos_sb[:, sc * QC:(sc + 1) * QC]
                sslice = sin_sb[:, sc * QC:(sc + 1) * QC]
                # rope in fp32 (bf16 only at the final q/k write):
                # out = raw*cos + swap32(raw)*sin_signed
                raw = rtmp.tile([128, QC], BF, name="raw", tag="raw")
                nc.vector.tensor_copy(raw[:], ps[:])
                rot = rtmp.tile([128, QC], BF, name="rot", tag="rot")
                for b32 in range(4):
                    src = (b32 ^ 1) * 32
                    nc.gpsimd.tensor_copy(rot[b32 * 32:(b32 + 1) * 32, :],
                                          raw[src:src + 32, :])
                t1 = rtmp.tile([128, QC], BF, name="t1", tag="t1")
                nc.vector.tensor_mul(t1[:], raw[:], cslice)
                nc.vector.tensor_mul(rot[:], rot[:], sslice)
                if rt < 4:
                    nc.vector.tensor_add(qT[rt][sc][:], t1[:], rot[:])
                else:
                    kr = rtmp.tile([128, QC], BF, name="kr", tag="kr")
                    nc.vector.tensor_add(kr[:], t1[:], rot[:])
                    for j in range(NKVL):
                        src = kr[j * 64:(j + 1) * 64, :]
                        nc.gpsimd.tensor_copy(kdup[j][sc][0:64, :], src)
                        nc.gpsimd.tensor_copy(kdup[j][sc][64:128, :], src)

            def v_tt(sc, tt):
                # V computed directly in natural (seq, dim) orientation:
                # lhsT = x.T seq-slice, rhs = wv.T -> out (seq, 2*64) + ones
                xt = xts[sc]
                kt = sc * 4 + tt
                vp = gps.tile([128, QC], F32, name="gp", tag="gp")
                for kd in range(NKD):
                    nc.tensor.matmul(vp[:, 0:128],
                                     xt[kd // 4][:, kd % 4, tt * 128:(tt + 1) * 128],
                                     wv_sb[:, kd, :],
                                     start=(kd == 0), stop=(kd == NKD - 1))
                for j in range(NKVL):
                    nc.vector.tensor_copy(vaug[kt][:, j, 0:64],
                                          vp[:, j * 64:(j + 1) * 64])
                    nc.gpsimd.memset(vaug[kt][:, j, 64:65], 1.0)

            def qkv_units(sc):
                if sc == 0:
                    # chunk 0 runs k and v first: their weights load first
                    # and attention phase 0 needs kdup/vaug anyway
                    units = [lambda: qkv_row(0, 4)]
                    units += [lambda tt=tt: v_tt(0, tt) for tt in range(4)]
                    units += [lambda rt=rt: qkv_row(0, rt) for rt in range(4)]
                    return units
                units = [lambda rt=rt: qkv_row(sc, rt) for rt in range(5)]
                units += [lambda tt=tt: v_tt(sc, tt) for tt in range(4)]
                return units

            # ---- wo partial + per-chunk ReduceScatter ----
            atiles = {}

            def wo_oc(i, oc):
                qc, q0, qn = SUBS[i]
                ps = gps.tile([128, QC], F32, name="gp", tag="gp")
                at = atiles[i]
                for j in range(4):
                    nc.tensor.matmul(ps[:, 0:qn],
                                     wo_sb[:, j, oc * 128:(oc + 1) * 128],
                                     at[:, j, 0:qn],
                                     start=(j == 0), stop=(j == 3))
                ot = opool.tile([128, QC], BF, name="ot", tag="ot")
                # PSUM is only readable by DVE/ACT; split the copies between
                # them (DVE-heavy: ACT is the softmax bottleneck)
                if oc % 3 == 2:
                    nc.scalar.copy(ot[:, 0:qn], ps[:, 0:qn])
                else:
                    nc.vector.tensor_copy(ot[:, 0:qn], ps[:, 0:qn])
                nc.sync.dma_start(rs_in[i][oc * 128:(oc + 1) * 128, 0:qn],
                                  ot[:, 0:qn])

            def rs_chunk(i):
                if _no_cc:
                    # sim-only: local copy instead of the collective, to
                    # measure compute-schedule quality without the cost
                    # model's collective pricing.
                    nc.gpsimd.dma_start(rs_out[i][:], rs_in[i][0:512, :])
                else:
                    nc.gpsimd.collective_compute(
                        "ReduceScatter", mybir.AluOpType.add,
                        replica_groups=[[0, 1, 2, 3], [4, 5, 6, 7]],
                        ins=[rs_in[i].opt()],
                        outs=[rs_out[i].opt()])
                if i != 4:
                    # the out-DMA is NOT issued here: on the in-order ACT
                    # queue it would wait for the RS to complete and
                    # head-of-line block the softmax exps for ~15-30us,
                    # starving PE.  out_dma(i) is instead queued as filler
                    # one chunk later, when the RS has long completed.
                    return
                if not _no_cc:
                    # teardown guard for the final block, whose reader was
                    # the only one ever observed stale: (1) ~22us of chained
                    # ACT busywork ahead of the out-DMA on the in-order ACT
                    # queue delays the rs_out[4] read well past the RS
                    # (delayed readers -- blocks 0-3 -- always see correct
                    # data); (2) a trailing dummy collective after the DMA
                    # ensures the real RS is never the last collective at
                    # program teardown.
                    d1 = guard.tile([128, 2048], F32, name="d1", tag="d1")
                    d2 = guard.tile([128, 2048], F32, name="d2", tag="d2")
                    g = nc.scalar.copy(d1[:], cos_sb[:])
                    if last_exp[0] is not None:
                        # without this gate the scheduler hoists the guard
                        # busywork into the attention region, starving the
                        # ACT engine exactly where exps are latency-critical
                        add_dep_helper(g.ins, last_exp[0], False,
                                       reason="guard chain after final exp")
                    for _ in range(6):
                        nc.scalar.copy(d2[:], d1[:])
                        nc.scalar.copy(d1[:], d2[:])
                out_dma(3)
                out_dma(4)
                if not _no_cc:
                    nc.gpsimd.collective_compute(
                        "AllGather", mybir.AluOpType.bypass,
                        replica_groups=[[0, 1, 2, 3], [4, 5, 6, 7]],
                        ins=[cg_in.opt()], outs=[cg_out.opt()])

            # most recently emitted exp activation; out_dma gates on it so
            # the tile scheduler cannot hoist the DMA (whose embedded
            # collective-semaphore wait would head-of-line block the ACT
            # queue) ahead of the attention exps emitted before it
            last_exp = [None]

            def out_dma(i):
                d = nc.scalar.dma_start(
                    out[rs_off[i]:rs_off[i] + 512 * rs_w[i]], rs_out[i][:])
                if last_exp[0] is not None:
                    add_dep_helper(d.ins, last_exp[0], False,
                                   reason="delay out-dma past attention exps")

            def wo_units(i):
                units = [lambda oc=oc: wo_oc(i, oc) for oc in range(16)]
                units.append(lambda i=i: rs_chunk(i))
                return units

            # ---- filler queue ----
            fq = deque()

            def drain(k):
                for _ in range(k):
                    if not fq:
                        return
                    fq.popleft()()

            def drain_all():
                while fq:
                    fq.popleft()()

            # ---- attention, software-pipelined over key tiles ----
            def attn_phase(i):
                qc, q0, qn = SUBS[i]
                gqs = qc * QC + q0          # 128-aligned global q start
                t0 = gqs // 128             # first diagonal key tile
                nkt = (gqs + qn) // 128     # causal: key tiles up to sub end
                # attention outputs staged in one tile: (128, rt, seq-chunk)
                atile = apool.tile([128, 4, QC], BF, name="atile", tag="atile")
                atiles[i] = atile
                for rt in range(4):  # head pair (2rt, 2rt+1); shared kv head
                    j = rt // 2
                    avs = [avps.tile([65, QC], F32, name="av", tag="av")
                           for _ in range(2)]
                    ptiles = {}

                    def emit_score(kt, rt=rt, j=j, ptiles=ptiles):
                        kb = (kt % 4) * 128
                        # diagonal k-tiles only need q columns >= 128*m
                        m = kt - t0
                        qo = 128 * m if m > 0 else 0
                        n = qn - qo
                        st = stps.tile([128, 2, QC], F32, name="st", tag="st")
                        for half in range(2):
                            # operands at partition base 64*half -> the two
                            # K=64 matmuls run in different PE row groups
                            lo, hi = half * 64, half * 64 + 64
                            nc.tensor.matmul(st[:, half, 0:n],
                                             kdup[j][kt // 4][lo:hi, kb:kb + 128],
                                             qT[rt][qc][lo:hi, q0 + qo:q0 + qn],
                                             start=True, stop=True)
                        p = ppool.tile([128, 2, QC], BF, name="p", tag="p")
                        e = nc.scalar.activation(p[:, :, 0:n], st[:, :, 0:n],
                                                 EXP, scale=SCALE)
                        last_exp[0] = e.ins if hasattr(e, "ins") else e
                        if m >= 0:
                            # only the first 128 q-columns of a diagonal tile
                            # intersect the triangle; the rest is unmasked
                            nc.vector.tensor_mul(p[:, :, 0:128], p[:, :, 0:128],
                                                 mask_sb[:])
                        ptiles[kt] = (p, qo, n)

                    def emit_av(kt, rt=rt, j=j, ptiles=ptiles, nkt=nkt):
                        p, qo, n = ptiles.pop(kt)
                        for half in range(2):
                            nc.tensor.matmul(avs[half][:, qo:qn],
                                             vaug[kt][:, j, :],
                                             p[:, half, 0:n],
                                             start=(kt == 0), stop=(kt == nkt - 1))

                    emit_score(0)
                    if nkt > 1:
                        emit_score(1)
                    credit = 0.0
                    for kt in range(nkt):
                        credit += cred_n * (qn - (128 * (kt - t0) if kt > t0 else 0)) / QC
                        while fq and (credit >= unit_ns or len(fq) > press):
                            fq.popleft()()
                            credit -= unit_ns
                        if kt + 2 < nkt:
                            emit_score(kt + 2)
                        emit_av(kt)
                    for half in range(2):
                        av = avs[half]
                        # drain PSUM->SBUF first so the av PSUM bank frees
                        # after one copy instead of after the whole
                        # recip->broadcast->mul chain (next rt's first AV
                        # matmul reuses the bank)
                        avsb = npool.tile([65, QC], BF, name="avsb",
                                          tag="avsb")
                        nc.vector.tensor_copy(avsb[:, 0:qn], av[:, 0:qn])
                        recip = npool.tile([1, QC], BF, name="recip", tag="recip")
                        nc.vector.reciprocal(recip[:, 0:qn], avsb[64:65, 0:qn])
                        rb = npool.tile([64, QC], BF, name="rb", tag="rb")
                        nc.gpsimd.partition_broadcast(rb[:, 0:qn], recip[:, 0:qn])
                        nc.vector.tensor_mul(
                            atile[half * 64:(half + 1) * 64, rt, 0:qn],
                            avsb[0:64, 0:qn], rb[:, 0:qn])
                    drain(1)

            # ---- schedule ----
            for u in qkv_units(0):
                u()
            fq.extend(qkv_units(1))
            attn_phase(0)
            fq.append(lambda: load_x(3))
            fq.extend(wo_units(0))
            fq.extend(qkv_units(2))
            attn_phase(1)
            fq.extend(wo_units(1))
            fq.extend(qkv_units(3))
            attn_phase(2)
            fq.extend(wo_units(2))
            fq.append(lambda: out_dma(0))
            attn_phase(3)
            # sub-3a wo runs immediately (not as paced filler) so its RS
            # dispatches before sub-3b attention and hides under it
            drain_all()
            for u in wo_units(3):
                u()
            fq.append(lambda: out_dma(1))
            attn_phase(4)
            drain_all()
            out_dma(2)
            for u in wo_units(4):
                u()

    nc.compile()
    return nc


def _get_nc():
    global _NC
    if _NC is None:
        _NC = _build()
    return _NC


def _prepare_in_maps(x, freqs_cis, wqkv, wo):
    x = np.asarray(x)
    freqs_cis = np.asarray(freqs_cis)
    wqkv = np.asarray(wqkv)
    wo = np.asarray(wo)

    perm = np.concatenate([np.arange(0, HD, 2), np.arange(1, HD, 2)])
    cos = np.ascontiguousarray(freqs_cis[:, :, 0].T)  # (32, S)
    sin = np.ascontiguousarray(freqs_cis[:, :, 1].T)
    cosS = np.ascontiguousarray(np.concatenate([cos, cos, cos, cos], axis=0),
                                dtype=np.float32).astype(BF16)
    sinS = np.ascontiguousarray(np.concatenate([-sin, sin, -sin, sin], axis=0),
                                dtype=np.float32).astype(BF16)
    p_i = np.arange(128)[:, None]
    f_i = np.arange(128)[None, :]
    tri = (f_i >= p_i)
    mask = np.stack([tri, tri], axis=1).astype(BF16)

    def stage(wt, nkd=NKD):
        # (Dc, C) with Dc = nkd*128 -> (128, nkd, C), per-partition contiguous
        return np.ascontiguousarray(
            wt.reshape(nkd, 128, wt.shape[1]).transpose(1, 0, 2)).astype(BF16)

    xSs = []
    for b in range(2):
        xt = x[b].T  # (D, S)
        xs = xt.reshape(NKD, 128, NSC, QC).transpose(2, 1, 0, 3)
        xSs.append(np.ascontiguousarray(xs).astype(BF16))

    in_maps = []
    for c in range(8):
        b, G = c // 4, c % 4
        qrows = np.concatenate([(8 * G + h) * HD + perm for h in range(NQL)])
        krows = np.concatenate([D + (2 * G + j) * HD + perm for j in range(NKVL)])
        vrows = np.concatenate([D + 512 + (2 * G + j) * HD + np.arange(HD)
                                for j in range(NKVL)])
        in_maps.append({
            "xS": xSs[b],
            "wqS": stage(wqkv[qrows, :].T),
            "wkS": stage(wqkv[krows, :].T),
            "wvS": stage(wqkv[vrows, :].T),
            "woS": stage(np.ascontiguousarray(wo[:, 512 * G:512 * (G + 1)].T),
                         nkd=4),
            "cosS": cosS,
            "sinS": sinS,
            "mask": mask,
        })
    return in_maps


def kernel(x, freqs_cis, wqkv, wo, _trace=False):
    in_maps = _prepare_in_maps(x, freqs_cis, wqkv, wo)
    res = run_bass_kernel_spmd(_get_nc(), in_maps, core_ids=list(range(8)),
                               trace=_trace)

    outf = np.empty((2, S, D), np.float32)
    # 5 rs blocks of (512 dims, w seq): seq chunks 0-2 full, chunk 3 halved
    blocks = [(0, 512), (512, 512), (1024, 512), (1536, 256), (1792, 256)]
    for c in range(8):
        b, G = c // 4, c % 4
        flat = np.asarray(res.results[c]["out"])
        off = 0
        for s0, w in blocks:
            blk = flat[off:off + 512 * w].reshape(512, w)
            outf[b, s0:s0 + w, 512 * G:512 * (G + 1)] = blk.T.astype(np.float32)
            off += 512 * w
    if _trace:
        kernel.last_exec_time_ns = res.exec_time_ns
        kernel.last_results = res
    return outf



# revision 39
# speedup vs baseline: 1.0138x; 1.0025x over previous
"""Distributed Trainium2 kernel for nn_Attention_68719477187.

RoPE + causal GQA attention (B=2, S=2048, DIM=2048, 32 q heads / 8 kv heads,
head_dim 64) on 8 NeuronCores: DP=2 over batch x TP=4 over head groups.

Per core (b = core//4, G = core%4): 8 q heads / 2 kv heads of batch b.
  1. qkv.T = w{q,k,v}T.T @ x_b.T (contraction over model dim on partitions)
  2. RoPE applied in transposed layout; head_dim pre-permuted (evens, odds)
     on the host so rotation pairs become contiguous 32-partition blocks.
  3. scores.T tiles (k on partitions, q on free) -> exp (no max subtraction;
     scores are O(5) so fp32 exp is safe) -> causal mask by 0/1 multiply on
     the 128-wide diagonal block only -> AV matmul with a ones-column
     appended to V so the softmax denominator falls out of the same matmul.
  4. wo partial sums: each core contracts its OWN 512 attention dims
     against wo rows for ALL 2048 output cols (no AllGather needed), then a
     per-seq-chunk ReduceScatter(add) over each batch group of 4 cores
     hands rank G the final 512-dim output block G.  The RS result lands
     directly in the bf16 output parameter (host transposes/casts).

Scheduling: the attention inner loop is software-pipelined (scores run two
key-tiles ahead of the AV matmuls) and a filler queue interleaves wo blocks
of the previous chunk and qkv rows of the next chunk between attention
tiles, keeping the PE tensor engine continuously busy (full p-state clock)
while the ACT engine works through the softmax exps.

Compute in bf16 (fp32 PSUM accumulation), output bf16 (cast on host).
"""

import sys

if "/opt/trn_rl_repo" not in sys.path:
    sys.path.insert(0, "/opt/trn_rl_repo")

from collections import deque

import numpy as np
import ml_dtypes

from concourse import bacc, tile, mybir
from concourse.bass_utils import run_bass_kernel_spmd
from concourse.tile_rust import add_dep_helper

BF16 = ml_dtypes.bfloat16

S = 2048          # sequence length
D = 2048          # model dim
HD = 64           # head dim
NQL = 8           # local q heads
NKVL = 2          # local kv heads
QC = 512          # q chunk (matmul free dim)
NSC = S // QC     # 4 seq chunks
NKD = D // 128    # 16 contraction tiles
NKT = S // 128    # 16 key tiles
SCALE = HD ** -0.5

# attention sub-chunks: (qT chunk, col offset, width).  Chunk 3 is split
# 256+256 so the 3a ReduceScatter (21.5us) hides under sub-3b's ~26us of
# attention PE work, leaving only the small 3b RS after PE finishes.
SUBS = [(0, 0, QC), (1, 0, QC), (2, 0, QC), (3, 0, 256), (3, 256, 256)]

_NC = None


def _build(_no_cc=False):
    import os
    # filler-drain tuning knobs (ns of filler credit per attention tile,
    # filler unit cost, queue pressure threshold); fall back to defaults.
    try:
        cred_n, unit_ns, press = [int(v) for v in
                                  os.environ.get("KTUNE", "").split(",")]
    except ValueError:
        cred_n, unit_ns, press = 650, 850, 26
    nc = bacc.Bacc("TRN2", target_bir_lowering=False, debug=False, num_devices=8)
    BF = mybir.dt.bfloat16
    F32 = mybir.dt.float32
    EXP = mybir.ActivationFunctionType.Exp

    # all inputs host-staged to per-partition-contiguous SBUF layouts so DMA
    # descriptor counts stay low (SEQ dispatch cost ~ descriptors)
    xS = nc.declare_dram_parameter("xS", [NSC, 128, NKD, QC], BF, isOutput=False)
    wqS = nc.declare_dram_parameter("wqS", [128, NKD, 512], BF, isOutput=False)
    wkS = nc.declare_dram_parameter("wkS", [128, NKD, 128], BF, isOutput=False)
    wvS = nc.declare_dram_parameter("wvS", [128, NKD, 128], BF, isOutput=False)
    woS = nc.declare_dram_parameter("woS", [128, 4, D], BF, isOutput=False)
    cosS = nc.declare_dram_parameter("cosS", [128, S], BF, isOutput=False)
    sinS = nc.declare_dram_parameter("sinS", [128, S], BF, isOutput=False)
    mask = nc.declare_dram_parameter("mask", [128, 2, 128], BF, isOutput=False)
    # flat output: 4 chunks of (512 out-dims, 512 seq) bf16, host reassembles
    out = nc.declare_dram_parameter("out", [NSC * 512 * QC], BF, isOutput=True)

    with tile.TileContext(nc) as tc:
        with (
            nc.allow_low_precision(reason="bf16 rope/softmax-normalize chain"),
            tc.tile_pool(name="wpool", bufs=1) as wpool,
            tc.tile_pool(name="pers", bufs=1) as pers,
            tc.tile_pool(name="dram", bufs=1, space="DRAM") as dram,
            tc.tile_pool(name="xpool", bufs=12) as xpool,
            tc.tile_pool(name="rtmp", bufs=4) as rtmp,
            tc.tile_pool(name="ppool", bufs=6) as ppool,
            tc.tile_pool(name="npool", bufs=3) as npool,
            tc.tile_pool(name="guard", bufs=1) as guard,
            tc.tile_pool(name="apool", bufs=2) as apool,
            tc.tile_pool(name="opool", bufs=6) as opool,
            tc.tile_pool(name="gps", bufs=2, space="PSUM") as gps,
            tc.tile_pool(name="stps", bufs=2, space="PSUM") as stps,
            tc.tile_pool(name="avps", bufs=2, space="PSUM") as avps,
        ):
            # ---- persistent weights / constants (one 3D DMA each) ----
            wq_sb = [wpool.tile([128, NKD // 4, 512], BF, name=f"wq_sb{h}",
                                tag=f"wq_sb{h}") for h in range(4)]
            wk_sb = wpool.tile([128, NKD, 128], BF, name="wk_sb", tag="wk_sb")
            wv_sb = wpool.tile([128, NKD, 128], BF, name="wv_sb", tag="wv_sb")
            wo_sb = wpool.tile([128, 4, D], BF, name="wo_sb", tag="wo_sb")
            cos_sb = wpool.tile([128, S], BF, name="cos_sb", tag="cos_sb")
            sin_sb = wpool.tile([128, S], BF, name="sin_sb", tag="sin_sb")
            mask_sb = wpool.tile([128, 2, 128], BF, name="mask_sb", tag="mask_sb")

            # ---- persistent activations ----
            qT = [[pers.tile([128, QC], BF, name=f"qT_{rt}_{sc}", tag=f"qT_{rt}_{sc}")
                   for sc in range(NSC)] for rt in range(4)]
            kdup = [[pers.tile([128, QC], BF, name=f"kd_{j}_{sc}", tag=f"kd_{j}_{sc}")
                     for sc in range(NSC)] for j in range(NKVL)]
            vaug = [pers.tile([128, 2, 65], BF, name=f"va_{kt}", tag=f"va_{kt}")
                    for kt in range(NKT)]
            # per-sub ReduceScatter staging: (2048 out-dims, qn seq); the
            # collective may not write IO tensors, so it lands in rs_out and
            # later DMAs move it to the out param.  Chunks 0-2 are whole;
            # chunk 3 is two 256-wide halves (subs 3 and 4).
            rs_w = [QC, QC, QC, 256, 256]
            rs_in = [dram.tile([D, rs_w[i]], BF, name=f"rs_in_{i}")
                     for i in range(5)]
            rs_out = [dram.tile([512, rs_w[i]], BF, name=f"rs_out_{i}")
                      for i in range(5)]
            # flat out offsets per rs block
            rs_off = [0, 512 * QC, 2 * 512 * QC, 3 * 512 * QC,
                      3 * 512 * QC + 512 * 256]

            # hoist x loads so the (in-order) SP DMA queue never stalls them;
            # wq / x chunk 0 are loaded in halves so the first matmuls start
            # after ~1MB of DMA instead of 4MB.
            xts = {}

            # the scheduler reorders same-queue DMAs by its own heuristics
            # (observed: chunk-1 x pushed behind wo/cos/sin, starving the
            # chunk-1 qkv fillers until ~40us); chain the startup preloads
            # with nosync deps so they issue in emission order
            prev_dma = [None]

            def sdma(dst, src):
                d = nc.sync.dma_start(dst, src)
                if prev_dma[0] is not None:
                    add_dep_helper(d.ins, prev_dma[0], False,
                                   reason="preload DMA order")
                prev_dma[0] = d.ins
                return d

            def load_x(sc, chain=False):
                parts = []
                for h in range(4):
                    xt = xpool.tile([128, NKD // 4, QC], BF, name="xt", tag="xt")
                    if chain:
                        sdma(xt[:], xS[sc, :, h * 4:(h + 1) * 4, :])
                    else:
                        nc.sync.dma_start(xt[:], xS[sc, :, h * 4:(h + 1) * 4, :])
                    parts.append(xt)
                xts[sc] = parts

            # k/v-first startup: wk+wv are small (0.5MB each) and the k row
            # and v tiles contract over x as its quarters arrive, so PE
            # starts ~4us in; wq quarters follow and the q rows run last
            sdma(wk_sb[:], wkS[:])
            xts[0] = []
            for h in range(4):
                xt = xpool.tile([128, NKD // 4, QC], BF, name="xt", tag="xt")
                sdma(xt[:], xS[0, :, h * 4:(h + 1) * 4, :])
                xts[0].append(xt)
                if h == 0:
                    sdma(wv_sb[:], wvS[:])
            sdma(cos_sb[:, 0:QC], cosS[:, 0:QC])
            sdma(sin_sb[:, 0:QC], sinS[:, 0:QC])
            sdma(mask_sb[:], mask[:])
            for h in range(4):
                sdma(wq_sb[h][:], wqS[:, h * 4:(h + 1) * 4, :])
            for sc in range(1, NSC - 1):
                load_x(sc, chain=True)
                sdma(cos_sb[:, sc * QC:(sc + 1) * QC],
                     cosS[:, sc * QC:(sc + 1) * QC])
                sdma(sin_sb[:, sc * QC:(sc + 1) * QC],
                     sinS[:, sc * QC:(sc + 1) * QC])
            sdma(cos_sb[:, 3 * QC:], cosS[:, 3 * QC:])
            sdma(sin_sb[:, 3 * QC:], sinS[:, 3 * QC:])
            sdma(wo_sb[:], woS[:])
            # scratch for the trailing dummy collective (teardown guard)
            cg_in = dram.tile([128, 2], BF, name="cg_in")
            cg_out = dram.tile([512, 2], BF, name="cg_out")
            nc.sync.dma_start(cg_in[:], mask[:, 0, 0:2])

            # ---- qkv projection + rope ----
            # each row is one atomic unit: its PSUM accumulation (tag "gp")
            # must not interleave with other "gp" allocations (buffer reuse
            # would clobber the accumulation in flight)
            def qkv_row(sc, rt):
                xt = xts[sc]
                ps = gps.tile([128, QC], F32, name="gp", tag="gp")
                for kd in range(NKD):
                    lhsT = (wq_sb[kd // 4][:, kd % 4, rt * 128:(rt + 1) * 128]
                            if rt < 4 else wk_sb[:, kd, :])
                    nc.tensor.matmul(ps[:], lhsT, xt[kd // 4][:, kd % 4, :],
                                     start=(kd == 0), stop=(kd == NKD - 1))
                cslice = cos_sb[:, sc * QC:(sc + 1) * QC]
                sslice = sin_sb[:, sc * QC:(sc + 1) * QC]
                # rope in fp32 (bf16 only at the final q/k write):
                # out = raw*cos + swap32(raw)*sin_signed
                raw = rtmp.tile([128, QC], BF, name="raw", tag="raw")
                nc.vector.tensor_copy(raw[:], ps[:])
                rot = rtmp.tile([128, QC], BF, name="rot", tag="rot")
                for b32 in range(4):
                    src = (b32 ^ 1) * 32
                    nc.gpsimd.tensor_copy(rot[b32 * 32:(b32 + 1) * 32, :],
                                          raw[src:src + 32, :])
                t1 = rtmp.tile([128, QC], BF, name="t1", tag="t1")
                nc.vector.tensor_mul(t1[:], raw[:], cslice)
                nc.vector.tensor_mul(rot[:], rot[:], sslice)
                if rt < 4:
                    nc.vector.tensor_add(qT[rt][sc][:], t1[:], rot[:])
                else:
                    kr = rtmp.tile([128, QC], BF, name="kr", tag="kr")
                    nc.vector.tensor_add(kr[:], t1[:], rot[:])
                    for j in range(NKVL):
                        src = kr[j * 64:(j + 1) * 64, :]
                        nc.gpsimd.tensor_copy(kdup[j][sc][0:64, :], src)
                        nc.gpsimd.tensor_copy(kdup[j][sc][64:128, :], src)

            def v_tt(sc, tt):
                # V computed directly in natural (seq, dim) orientation:
                # lhsT = x.T seq-slice, rhs = wv.T -> out (seq, 2*64) + ones
                xt = xts[sc]
                kt = sc * 4 + tt
                vp = gps.tile([128, QC], F32, name="gp", tag="gp")
                for kd in range(NKD):
                    nc.tensor.matmul(vp[:, 0:128],
                                     xt[kd // 4][:, kd % 4, tt * 128:(tt + 1) * 128],
                                     wv_sb[:, kd, :],
                                     start=(kd == 0), stop=(kd == NKD - 1))
                for j in range(NKVL):
                    nc.vector.tensor_copy(vaug[kt][:, j, 0:64],
                                          vp[:, j * 64:(j + 1) * 64])
                    nc.gpsimd.memset(vaug[kt][:, j, 64:65], 1.0)

            def qkv_units(sc):
                if sc == 0:
                    # chunk 0 runs k and v first: their weights load first
                    # and attention phase 0 needs kdup/vaug anyway
                    units = [lambda: qkv_row(0, 4)]
                    units += [lambda tt=tt: v_tt(0, tt) for tt in range(4)]
                    units += [lambda rt=rt: qkv_row(0, rt) for rt in range(4)]
                    return units
                units = [lambda rt=rt: qkv_row(sc, rt) for rt in range(5)]
                units += [lambda tt=tt: v_tt(sc, tt) for tt in range(4)]
                return units

            # ---- wo partial + per-chunk ReduceScatter ----
            atiles = {}

            def wo_oc(i, oc):
                qc, q0, qn = SUBS[i]
                ps = gps.tile([128, QC], F32, name="gp", tag="gp")
                at = atiles[i]
                for j in range(4):
                    nc.tensor.matmul(ps[:, 0:qn],
                                     wo_sb[:, j, oc * 128:(oc + 1) * 128],
                                     at[:, j, 0:qn],
                                     start=(j == 0), stop=(j == 3))
                ot = opool.tile([128, QC], BF, name="ot", tag="ot")
                # PSUM is only readable by DVE/ACT; split the copies between
                # them (DVE-heavy: ACT is the softmax bottleneck)
                if oc % 3 == 2:
                    nc.scalar.copy(ot[:, 0:qn], ps[:, 0:qn])
                else:
                    nc.vector.tensor_copy(ot[:, 0:qn], ps[:, 0:qn])
                nc.sync.dma_start(rs_in[i][oc * 128:(oc + 1) * 128, 0:qn],
                                  ot[:, 0:qn])

            def rs_chunk(i):
                if _no_cc:
                    # sim-only: local copy instead of the collective, to
                    # measure compute-schedule quality without the cost
                    # model's collective pricing.
                    nc.gpsimd.dma_start(rs_out[i][:], rs_in[i][0:512, :])
                else:
                    cc = nc.gpsimd.collective_compute(
                        "ReduceScatter", mybir.AluOpType.add,
                        replica_groups=[[0, 1, 2, 3], [4, 5, 6, 7]],
                        ins=[rs_in[i].opt()],
                        outs=[rs_out[i].opt()])
                    last_rs[0] = cc.ins if hasattr(cc, "ins") else cc
                if i != 4:
                    # the out-DMA is NOT issued here: on the in-order ACT
                    # queue it would wait for the RS to complete and
                    # head-of-line block the softmax exps for ~15-30us,
                    # starving PE.  out_dma(i) is instead queued as filler
                    # one chunk later, when the RS has long completed.
                    return
                out_dma(3)
                if not _no_cc:
                    # stale-read guard for the final block: the collective's
                    # completion semaphore can fire before its DRAM writes
                    # are visible (observed: razor-edge readers of the last
                    # RS get garbage/NaN).  Two ACT copies sync-gated on the
                    # RS3b instruction insert a guaranteed ~2us of physical
                    # delay between the semaphore and the rs_out[4] read,
                    # robust to any schedule shift.  The trailing dummy
                    # collective keeps the real RS from being the last
                    # collective at program teardown.
                    d1 = guard.tile([128, 2048], F32, name="d1", tag="d1")
                    g = nc.scalar.copy(d1[:, 0:1024], cos_sb[:, 0:1024])
                    if last_rs[0] is not None:
                        add_dep_helper(g.ins, last_rs[0], True,
                                       reason="guard delay after final RS")
                    if last_exp[0] is not None:
                        add_dep_helper(g.ins, last_exp[0], False,
                                       reason="keep guard out of attention")
                    nc.scalar.copy(d1[:, 1024:2048], cos_sb[:, 1024:2048])
                out_dma(4)
                if not _no_cc:
                    nc.gpsimd.collective_compute(
                        "AllGather", mybir.AluOpType.bypass,
                        replica_groups=[[0, 1, 2, 3], [4, 5, 6, 7]],
                        ins=[cg_in.opt()], outs=[cg_out.opt()])

            last_rs = [None]
            # most recently emitted exp activation; out_dma gates on it so
            # the tile scheduler cannot hoist the DMA (whose embedded
            # collective-semaphore wait would head-of-line block the ACT
            # queue) ahead of the attention exps emitted before it
            last_exp = [None]

            def out_dma(i):
                d = nc.scalar.dma_start(
                    out[rs_off[i]:rs_off[i] + 512 * rs_w[i]], rs_out[i][:])
                if last_exp[0] is not None:
                    add_dep_helper(d.ins, last_exp[0], False,
                                   reason="delay out-dma past attention exps")

            def wo_units(i):
                units = [lambda oc=oc: wo_oc(i, oc) for oc in range(16)]
                units.append(lambda i=i: rs_chunk(i))
                return units

            # ---- filler queue ----
            fq = deque()

            def drain(k):
                for _ in range(k):
                    if not fq:
                        return
                    fq.popleft()()

            def drain_all():
                while fq:
                    fq.popleft()()

            # ---- attention, software-pipelined over key tiles ----
            def attn_phase(i):
                qc, q0, qn = SUBS[i]
                gqs = qc * QC + q0          # 128-aligned global q start
                t0 = gqs // 128             # first diagonal key tile
                nkt = (gqs + qn) // 128     # causal: key tiles up to sub end
                # attention outputs staged in one tile: (128, rt, seq-chunk)
                atile = apool.tile([128, 4, QC], BF, name="atile", tag="atile")
                atiles[i] = atile
                for rt in range(4):  # head pair (2rt, 2rt+1); shared kv head
                    j = rt // 2
                    avs = [avps.tile([65, QC], F32, name="av", tag="av")
                           for _ in range(2)]
                    ptiles = {}

                    def emit_score(kt, rt=rt, j=j, ptiles=ptiles):
                        kb = (kt % 4) * 128
                        # diagonal k-tiles only need q columns >= 128*m
                        m = kt - t0
                        qo = 128 * m if m > 0 else 0
                        n = qn - qo
                        st = stps.tile([128, 2, QC], F32, name="st", tag="st")
                        for half in range(2):
                            # operands at partition base 64*half -> the two
                            # K=64 matmuls run in different PE row groups
                            lo, hi = half * 64, half * 64 + 64
                            nc.tensor.matmul(st[:, half, 0:n],
                                             kdup[j][kt // 4][lo:hi, kb:kb + 128],
                                             qT[rt][qc][lo:hi, q0 + qo:q0 + qn],
                                             start=True, stop=True)
                        p = ppool.tile([128, 2, QC], BF, name="p", tag="p")
                        e = nc.scalar.activation(p[:, :, 0:n], st[:, :, 0:n],
                                                 EXP, scale=SCALE)
                        last_exp[0] = e.ins if hasattr(e, "ins") else e
                        if m >= 0:
                            # only the first 128 q-columns of a diagonal tile
                            # intersect the triangle; the rest is unmasked
                            nc.vector.tensor_mul(p[:, :, 0:128], p[:, :, 0:128],
                                                 mask_sb[:])
                        ptiles[kt] = (p, qo, n)

                    def emit_av(kt, rt=rt, j=j, ptiles=ptiles, nkt=nkt):
                        p, qo, n = ptiles.pop(kt)
                        for half in range(2):
                            nc.tensor.matmul(avs[half][:, qo:qn],
                                             vaug[kt][:, j, :],
                                             p[:, half, 0:n],
                                             start=(kt == 0), stop=(kt == nkt - 1))

                    emit_score(0)
                    if nkt > 1:
                        emit_score(1)
                    credit = 0.0
                    for kt in range(nkt):
                        credit += cred_n * (qn - (128 * (kt - t0) if kt > t0 else 0)) / QC
                        while fq and (credit >= unit_ns or len(fq) > press):
                            fq.popleft()()
                            credit -= unit_ns
                        if kt + 2 < nkt:
                            emit_score(kt + 2)
                        emit_av(kt)
                    for half in range(2):
                        av = avs[half]
                        # drain PSUM->SBUF first so the av PSUM bank frees
                        # after one copy instead of after the whole
                        # recip->broadcast->mul chain (next rt's first AV
                        # matmul reuses the bank)
                        avsb = npool.tile([65, QC], BF, name="avsb",
                                          tag="avsb")
                        nc.vector.tensor_copy(avsb[:, 0:qn], av[:, 0:qn])
                        recip = npool.tile([1, QC], BF, name="recip", tag="recip")
                        nc.vector.reciprocal(recip[:, 0:qn], avsb[64:65, 0:qn])
                        rb = npool.tile([64, QC], BF, name="rb", tag="rb")
                        nc.gpsimd.partition_broadcast(rb[:, 0:qn], recip[:, 0:qn])
                        nc.vector.tensor_mul(
                            atile[half * 64:(half + 1) * 64, rt, 0:qn],
                            avsb[0:64, 0:qn], rb[:, 0:qn])
                    drain(1)

            # ---- schedule ----
            for u in qkv_units(0):
                u()
            fq.extend(qkv_units(1))
            attn_phase(0)
            fq.append(lambda: load_x(3))
            fq.extend(wo_units(0))
            fq.extend(qkv_units(2))
            attn_phase(1)
            fq.extend(wo_units(1))
            fq.extend(qkv_units(3))
            attn_phase(2)
            fq.extend(wo_units(2))
            fq.append(lambda: out_dma(0))
            attn_phase(3)
            # sub-3a wo runs immediately (not as paced filler) so its RS
            # dispatches before sub-3b attention and hides under it
            drain_all()
            for u in wo_units(3):
                u()
            fq.append(lambda: out_dma(1))
            attn_phase(4)
            drain_all()
            out_dma(2)
            for u in wo_units(4):
                u()

    nc.compile()
    return nc


def _get_nc():
    global _NC
    if _NC is None:
        _NC = _build()
    return _NC


def _prepare_in_maps(x, freqs_cis, wqkv, wo):
    x = np.asarray(x)
    freqs_cis = np.asarray(freqs_cis)
    wqkv = np.asarray(wqkv)
    wo = np.asarray(wo)

    perm = np.concatenate([np.arange(0, HD, 2), np.arange(1, HD, 2)])
    cos = np.ascontiguousarray(freqs_cis[:, :, 0].T)  # (32, S)
    sin = np.ascontiguousarray(freqs_cis[:, :, 1].T)
    cosS = np.ascontiguousarray(np.concatenate([cos, cos, cos, cos], axis=0),
                                dtype=np.float32).astype(BF16)
    sinS = np.ascontiguousarray(np.concatenate([-sin, sin, -sin, sin], axis=0),
                                dtype=np.float32).astype(BF16)
    p_i = np.arange(128)[:, None]
    f_i = np.arange(128)[None, :]
    tri = (f_i >= p_i)
    mask = np.stack([tri, tri], axis=1).astype(BF16)

    def stage(wt, nkd=NKD):
        # (Dc, C) with Dc = nkd*128 -> (128, nkd, C), per-partition contiguous
        return np.ascontiguousarray(
            wt.reshape(nkd, 128, wt.shape[1]).transpose(1, 0, 2)).astype(BF16)

    xSs = []
    for b in range(2):
        xt = x[b].T  # (D, S)
        xs = xt.reshape(NKD, 128, NSC, QC).transpose(2, 1, 0, 3)
        xSs.append(np.ascontiguousarray(xs).astype(BF16))

    in_maps = []
    for c in range(8):
        b, G = c // 4, c % 4
        qrows = np.concatenate([(8 * G + h) * HD + perm for h in range(NQL)])
        krows = np.concatenate([D + (2 * G + j) * HD + perm for j in range(NKVL)])
        vrows = np.concatenate([D + 512 + (2 * G + j) * HD + np.arange(HD)
                                for j in range(NKVL)])
        in_maps.append({
            "xS": xSs[b],
            "wqS": stage(wqkv[qrows, :].T),
            "wkS": stage(wqkv[krows, :].T),
            "wvS": stage(wqkv[vrows, :].T),
            "woS": stage(np.ascontiguousarray(wo[:, 512 * G:512 * (G + 1)].T),
                         nkd=4),
            "cosS": cosS,
            "sinS": sinS,
            "mask": mask,
        })
    return in_maps


def kernel(x, freqs_cis, wqkv, wo, _trace=False):
    in_maps = _prepare_in_maps(x, freqs_cis, wqkv, wo)
    res = run_bass_kernel_spmd(_get_nc(), in_maps, core_ids=list(range(8)),
                               trace=_trace)

    outf = np.empty((2, S, D), np.float32)
    # 5 rs blocks of (512 dims, w seq): seq chunks 0-2 full, chunk 3 halved
    blocks = [(0, 512), (512, 512), (1024, 512), (1536, 256), (1792, 256)]
    for c in range(8):
        b, G = c // 4, c % 4
        flat = np.asarray(res.results[c]["out"])
        off = 0
        for s0, w in blocks:
            blk = flat[off:off + 512 * w].reshape(512, w)
            outf[b, s0:s0 + w, 512 * G:512 * (G + 1)] = blk.T.astype(np.float32)
            off += 512 * w
    if _trace:
        kernel.last_exec_time_ns = res.exec_time_ns
        kernel.last_results = res
    return outf

